# revision 23
# baseline (speedup 1.0000x reference)
"""SMPL body-model (B=512, V=6890, J=24) Bass kernel for 8 Trainium2 cores.

Strategy: vertex-shard V across the 8 cores (864 verts/core, zero-padded to
6912); every core computes the full batch B=512 for the small per-batch work
(Rodrigues, joint regression, kinematic chain) and its vertex slice for the
heavy per-vertex work.

Key algebraic restructure (avoids materializing [B,V,4,4] skinning mats):
  verts[b,v,m] = sum_{j,n} rel[b,j,m,n] * G[(n,j), v]              (term1)
               + sum_{j,n<3} rel[b,j,m,n] * w[v,j] * delta[b,v,n]  (corr)
  G[(n,j), v]  = w[v,j] * vt_h[n, v]   (batch-independent, host precomputed)
  delta[b,v,:] = blend-shape offsets + pose offsets (small magnitude)
so term1 is one K=96 fp32 matmul per output tile, and the correction runs
through K=24 matmuls (M3) + a per-vertex 4-wide multiply-reduce on DVE.
delta / M3 matmuls run as float32r (small-magnitude corrections only).
"""

import os
import numpy as np

B, V, NJ, NB = 512, 6890, 24, 10
NCORES = 8
VS = 864                    # vertex slice per core (8*864 = 6912 >= 6890)
VC = 432                    # vertex chunk (psum-bank aligned work unit)
BG = 4                      # batch groups of 128
PAR = [-1, 0, 0, 0, 1, 2, 3, 4, 5, 6, 7, 8, 9, 9, 9,
       12, 13, 14, 16, 17, 18, 19, 20, 21]

# matmul dtype for the correction path: "f32r" | "bf16" | "f32"
CORR_DT = os.environ.get("BODY_CORR_DT", "f32r")
STAGE = int(os.environ.get("BODY_STAGE", "99"))
DEBUG = bool(int(os.environ.get("BODY_DEBUG", "0")))
# dtype of the mul-reduce intermediate: bf16 (fast) or f32 (exact)
TMP_DT = os.environ.get("BODY_TMP_DT", "bf16")

_CACHE = {}


def _emit(nc, tc, tens):
    import concourse.bass as bass
    import concourse.mybir as mybir
    from concourse.bass import MemorySpace

    dt = mybir.dt
    f32 = dt.float32
    bf16 = dt.bfloat16
    AF = mybir.ActivationFunctionType
    ALU = mybir.AluOpType

    # dtype for correction-path matmul operand tiles; fp32r tiles must be
    # *produced* as fp32r (BIR verifier), so the tiles are declared f32r
    # and fp32 DRAM sources are bitcast at the DMA.
    corr_sb_dt = {"f32r": dt.float32r, "bf16": bf16, "f32": f32}[CORR_DT]

    def corr_src(ap):
        if CORR_DT == "f32r":
            return ap.bitcast(dt.float32r)
        return ap

    import contextlib
    stk = contextlib.ExitStack()
    cpool = stk.enter_context(tc.tile_pool(name="consts", bufs=1))
    wpool = stk.enter_context(tc.tile_pool(name="work", bufs=1))
    ppool = stk.enter_context(tc.tile_pool(name="ps", bufs=2, space="PSUM"))
    spool = stk.enter_context(tc.tile_pool(name="stream", bufs=3))

    # ---- load constants ----
    def load(name, shape, dtype=f32):
        t = cpool.tile(shape, dtype, tag=name)
        srcap = tens[name][:]
        if dtype == dt.float32r:
            srcap = srcap.bitcast(dt.float32r)
        nc.sync.dma_start(t[:], srcap)
        return t

    posef = load("pose_t", [128, BG, NJ, 3])
    bt = load("btaug", [NB + 1, B])
    btr = cpool.tile([NB, B], corr_sb_dt, tag="btr")
    nc.sync.dma_start(btr[:], corr_src(tens["btaug"][0:NB, :]))
    jd = load("jdirs", [NB + 1, NJ * 3])
    negIh = load("negI_hi", [126, 1])
    negIl = load("negI_lo", [81, 1])
    ident = load("ident", [128, 128])
    G = load("G", [128, VS])
    wT = load("wT", [88, VS], corr_sb_dt)
    pdh = load("pd_hi", [126, VS * 3], corr_sb_dt)
    pdl = load("pd_lo", [81, VS * 3], corr_sb_dt)
    sdt = load("sdT", [NB, VS * 3], corr_sb_dt)

    verts_out = tens["verts_out"]
    joints_out = tens["joints_out"]

    if STAGE < 1:
        stk.close()
        return
    # ---- Rodrigues: pose [128,(bg,j,3)] -> R [128,(bg,j,3,3)] ----
    rv1 = wpool.tile([128, BG, NJ, 3], f32)
    nc.vector.tensor_scalar_add(rv1[:], posef[:], 1e-8)
    sq = wpool.tile([128, BG, NJ, 3], f32)
    nc.scalar.square(sq[:], rv1[:])
    th2 = wpool.tile([128, BG, NJ], f32)
    nc.vector.reduce_sum(th2[:], sq[:], axis=mybir.AxisListType.X)
    th = wpool.tile([128, BG, NJ], f32)
    nc.scalar.sqrt(th[:], th2[:])
    rth = wpool.tile([128, BG, NJ], f32)
    nc.vector.reciprocal(rth[:], th[:])
    sin = wpool.tile([128, BG, NJ], f32)
    nc.scalar.activation(sin[:], th[:], AF.Sin)
    thc = wpool.tile([128, BG, NJ], f32)
    nc.vector.tensor_scalar_add(thc[:], th[:], float(np.pi / 2))
    cos = wpool.tile([128, BG, NJ], f32)
    nc.scalar.activation(cos[:], thc[:], AF.Sin)
    omc = wpool.tile([128, BG, NJ], f32)
    nc.scalar.activation(omc[:], cos[:], AF.Identity, bias=1.0, scale=-1.0)

    if STAGE < 1.5 and STAGE >= 1:
        pass
    rd = wpool.tile([128, BG, NJ, 3], f32)
    nc.vector.tensor_mul(rd[:], posef[:],
                         rth[:].unsqueeze(3).broadcast_to([128, BG, NJ, 3]))
    srd = wpool.tile([128, BG, NJ, 3], f32)
    nc.vector.tensor_mul(srd[:], rd[:],
                         sin[:].unsqueeze(3).broadcast_to([128, BG, NJ, 3]))
    # R = omc * (rd rd^T);  then += cos on diag, += sin*K off-diag
    ord_t = wpool.tile([128, BG, NJ, 3], f32)
    nc.vector.tensor_mul(ord_t[:], rd[:],
                         omc[:].unsqueeze(3).broadcast_to([128, BG, NJ, 3]))
    R = wpool.tile([128, BG, NJ, 3, 3], f32)
    nc.vector.tensor_mul(
        R[:],
        rd[:].unsqueeze(4).broadcast_to([128, BG, NJ, 3, 3]),
        ord_t[:].unsqueeze(3).broadcast_to([128, BG, NJ, 3, 3]))
    Rf = R[:].rearrange("p g j m n -> p g j (m n)")
    diag = Rf[:, :, :, 0:9:4]
    nc.vector.tensor_add(diag, diag,
                         cos[:].unsqueeze(3).broadcast_to([128, BG, NJ, 3]))
    # K matrix: [[0,-z,y],[z,0,-x],[-y,x,0]] * sin
    for (mn, comp, sign) in ((1, 2, -1.0), (3, 2, 1.0), (2, 1, 1.0),
                             (6, 1, -1.0), (5, 0, -1.0), (7, 0, 1.0)):
        dst = Rf[:, :, :, mn:mn + 1].squeeze(3)
        src = srd[:, :, :, comp:comp + 1].squeeze(3)
        if sign > 0:
            nc.vector.tensor_add(dst, dst, src)
        else:
            nc.vector.tensor_sub(dst, dst, src)

    if STAGE < 2:
        stk.close()
        return
    if DEBUG:
        nc.sync.dma_start(tens["dbg_R"][:],
                          R[:].rearrange("p g j m n -> p (g j m n)"))

    # ---- pose-feature transposes: pfh [126, B], pfl [81, B] (rows (j,m,n)) ----
    pfh = wpool.tile([126, B], corr_sb_dt)
    pfl = wpool.tile([81, B], corr_sb_dt)
    pf_src = R[:].rearrange("p g j m n -> p g (j m n)")
    for bg in range(BG):
        for (lo, cnt, dstt, nI) in ((0, 126, pfh, negIh), (126, 81, pfl, negIl)):
            tp = ppool.tile([cnt, 128], f32, tag="ps")
            nc.tensor.transpose(
                tp[:], pf_src[:, bg, 9 + lo: 9 + lo + cnt], ident[:])
            nc.scalar.activation(
                dstt[:, bg * 128:(bg + 1) * 128], tp[:], AF.Identity,
                bias=nI[:, 0:1])

    if STAGE < 2:
        stk.close()
        return
    # ---- J = betas_aug @ jdirs_aug -> [128,(bg,j,3)] ----
    J = wpool.tile([128, BG, NJ, 3], f32)
    for bg in range(BG):
        jp = ppool.tile([128, NJ * 3], f32, tag="ps")
        nc.tensor.matmul(jp[:], bt[:, bg * 128:(bg + 1) * 128], jd[:])
        nc.scalar.copy(J[:, bg][:].rearrange("p j k -> p (j k)"), jp[:])

    if STAGE < 3:
        stk.close()
        return
    if DEBUG:
        nc.sync.dma_start(tens["dbg_pfh"][:], pfh[:])

    # ---- delta matmuls -> d4 [128,(bg,v,4)] (n=3 unused pad) ----
    d4 = wpool.tile([128, BG, VS, 4], f32)
    if DEBUG:
        nc.gpsimd.memset(d4[:, :, :, 3:4], 1.0)
    for bg in range(BG):
        bsl = slice(bg * 128, (bg + 1) * 128)
        for vc in range(VS // VC):
            dp = ppool.tile([128, 3, 512], f32, tag="ps")
            for ch in range(3):
                csl = slice(vc * 3 * VC + ch * VC, vc * 3 * VC + (ch + 1) * VC)
                mms = ((pfh, pdh), (pfl, pdl), (btr, sdt))
                for i, (lhs, rhs) in enumerate(mms):
                    nc.tensor.matmul(
                        dp[:, ch, 0:VC], lhs[:, bsl], rhs[:, csl],
                        start=(i == 0), stop=(i == len(mms) - 1))
            # psum cols are (v,n)-interleaved: chunk ch = verts [ch*144,+144)
            nc.scalar.copy(
                d4[:, bg, vc * VC:(vc + 1) * VC, 0:3]
                .rearrange("p (c v) n -> p c v n", c=3),
                dp[:, :, 0:VC].rearrange("p c (v n) -> p c v n", n=3))

    if STAGE < 4:
        stk.close()
        return
    if DEBUG:
        nc.sync.dma_start(tens["dbg_J"][:], J[:].rearrange("p g j n -> p (g j n)"))
        nc.sync.dma_start(tens["dbg_d4"][:],
                          d4[:].rearrange("p g v n -> p (g v n)"))

    # ---- rel_joints (per kinematic-tree runs) ----
    rj = wpool.tile([128, BG, NJ, 3], f32)
    nc.vector.tensor_copy(rj[:, :, 0], J[:, :, 0])
    nc.vector.tensor_sub(rj[:, :, 1:4], J[:, :, 1:4],
                         J[:, :, 0:1].broadcast_to([128, BG, 3, 3]))
    nc.vector.tensor_sub(rj[:, :, 4:13], J[:, :, 4:13], J[:, :, 1:10])
    nc.vector.tensor_sub(rj[:, :, 13:15], J[:, :, 13:15],
                         J[:, :, 9:10].broadcast_to([128, BG, 2, 3]))
    nc.vector.tensor_sub(rj[:, :, 15:18], J[:, :, 15:18], J[:, :, 12:15])
    nc.vector.tensor_sub(rj[:, :, 18:24], J[:, :, 18:24], J[:, :, 16:22])

    if STAGE < 5:
        stk.close()
        return
    # ---- tmat: [R | rel_joints] 3x4 ----
    tm = wpool.tile([128, BG, NJ, 3, 4], f32)
    for bg in range(BG):
        nc.vector.tensor_copy(tm[:, bg, :, :, 0:3], R[:, bg])
    nc.vector.tensor_copy(tm[:, :, :, :, 3], rj[:])

    # ---- kinematic chain (3x4 affine composition) ----
    # j padded to 32 so the (n, j) transpose lands n-blocks at 32-aligned
    # partitions (matmul operand base must be 0/32/64); pad rows stay zero.
    ch_t = wpool.tile([128, BG, 32, 3, 4], f32)
    nc.gpsimd.memset(ch_t[:], 0.0)
    nc.vector.tensor_copy(ch_t[:, :, 0, :, :], tm[:, :, 0])
    tmpj = wpool.tile([128, BG, 3, 4, 3], f32)
    for j in range(1, NJ):
        p = PAR[j]
        for bg in range(BG):
            in0 = (ch_t[:, bg, p, :, 0:3].unsqueeze(2)
                   .broadcast_to([128, 3, 4, 3]))
            in1 = (tm[:, bg, j].unsqueeze(1).transpose([0, 1, 3, 2])
                   .broadcast_to([128, 3, 4, 3]))
            nc.vector.tensor_mul(tmpj[:, bg], in0, in1)
            nc.vector.reduce_sum(ch_t[:, bg, j, :, :], tmpj[:, bg],
                                 axis=mybir.AxisListType.X)
        nc.vector.tensor_add(ch_t[:, :, j, :, 3], ch_t[:, :, j, :, 3],
                             ch_t[:, :, p, :, 3])

    if STAGE < 6:
        stk.close()
        return
    # posed joints output (before the rel subtraction)
    jstage = wpool.tile([128, BG, NJ, 3], f32)
    nc.vector.tensor_copy(jstage[:], ch_t[:, :, 0:NJ, :, 3])
    nc.sync.dma_start(
        joints_out[:].rearrange("p g (j k) -> p g j k", k=3), jstage[:])

    if STAGE < 7:
        stk.close()
        return
    # ---- rel transforms: translation -= chain_rot @ J ----
    tj_t = wpool.tile([128, BG, NJ, 3, 3], f32)
    tj = wpool.tile([128, BG, NJ, 3], f32)
    for bg in range(BG):
        nc.vector.tensor_mul(
            tj_t[:, bg], ch_t[:, bg, 0:NJ, :, 0:3],
            J[:, bg].unsqueeze(2).broadcast_to([128, NJ, 3, 3]))
        nc.vector.reduce_sum(tj[:, bg], tj_t[:, bg],
                             axis=mybir.AxisListType.X)
    nc.vector.tensor_sub(ch_t[:, :, 0:NJ, :, 3], ch_t[:, :, 0:NJ, :, 3], tj[:])

    if STAGE < 8:
        stk.close()
        return
    if DEBUG:
        nc.sync.dma_start(tens["dbg_ch"][:],
                          ch_t[:].rearrange("p g j m n -> p (g j m n)"))

    # ---- transposes -> relT1 [128=(n,j32), 12=(m,bg), 128] ----
    relT1 = wpool.tile([128, 12, 128], f32)
    relT1b = (wpool.tile([128, 12, 128], corr_sb_dt, tag="relT1b",
                         name="relT1b")
              if CORR_DT != "f32" else None)
    for m in range(3):
        for bg in range(BG):
            # contiguous (n, j) staging: transpose moving op needs 1 free dim
            cstg = spool.tile([128, 128], f32, tag="cstg")
            nc.vector.tensor_copy(
                cstg[:].rearrange("p (n j) -> p n j", n=4),
                ch_t[:, bg, :, m, :].transpose([0, 2, 1]))
            tp = ppool.tile([128, 128], f32, tag="ps")
            nc.tensor.transpose(tp[:], cstg[:], ident[:])
            nc.scalar.copy(relT1[:, m * 4 + bg], tp[:])
            if relT1b is not None:
                nc.scalar.copy(relT1b[:, m * 4 + bg], tp[:])

    if STAGE < 9:
        stk.close()
        return
    if DEBUG:
        nc.sync.dma_start(tens["dbg_relT1"][:],
                          relT1[:].rearrange("p c b -> p (c b)"))

    # ---- mv units: per (m, bg, vchunk): 3 M3 matmuls + term1 + mul-reduce ----
    relT3 = relT1b if relT1b is not None else relT1
    for m in range(3):
        for bg in range(BG):
            c = m * 4 + bg
            for vc in range(VS // VC):
                vsl = slice(vc * VC, (vc + 1) * VC)
                mv = ppool.tile([128, 4, 512], f32, tag="ps")
                for n in range(3):
                    nc.tensor.matmul(
                        mv[:, n, 0:VC],
                        relT3[n * 32:n * 32 + NJ, c],
                        wT[n * 32:n * 32 + NJ, vsl])
                nc.tensor.matmul(mv[:, 3, 0:VC], relT1[:, c], G[:, vsl])
                tmpb = spool.tile([128, VC, 3],
                                  bf16 if TMP_DT == "bf16" else f32, tag="tmpb")
                nc.vector.tensor_mul(
                    tmpb[:], mv[:, 0:3, 0:VC].transpose([0, 2, 1]),
                    d4[:, bg, vsl, 0:3])
                vout = spool.tile([128, VC], f32, tag="vout")
                nc.vector.reduce_sum(vout[:], tmpb[:], axis=mybir.AxisListType.X)
                nc.vector.tensor_add(vout[:], vout[:], mv[:, 3, 0:VC])
                nc.sync.dma_start(verts_out[c, :, vsl], vout[:])

    stk.close()


def _build():
    key = ("nc", CORR_DT, TMP_DT)
    if key in _CACHE:
        return _CACHE[key]
    import concourse.bacc as bacc
    import concourse.tile as tile
    import concourse.mybir as mybir

    dt = mybir.dt
    f32 = dt.float32
    cbdt = dt.bfloat16 if CORR_DT == "bf16" else f32

    nc = bacc.Bacc("TRN2", target_bir_lowering=False, debug=False)
    tens = {}

    def din(name, shape, dtype=f32):
        tens[name] = nc.dram_tensor(name, shape, dtype, kind="ExternalInput")[:]

    din("pose_t", [128, BG, NJ, 3])
    din("btaug", [NB + 1, B])
    din("jdirs", [NB + 1, NJ * 3])
    din("negI_hi", [126, 1])
    din("negI_lo", [81, 1])
    din("ident", [128, 128])
    din("G", [128, VS])
    din("wT", [88, VS], cbdt)
    din("pd_hi", [126, VS * 3], cbdt)
    din("pd_lo", [81, VS * 3], cbdt)
    din("sdT", [NB, VS * 3], cbdt)
    tens["verts_out"] = nc.dram_tensor(
        "verts_out", [12, 128, VS], f32, kind="ExternalOutput")[:]
    if os.environ.get("BODY_DEBUG", "0") != "0":
        for nm, shp in (("dbg_R", [128, 4 * NJ * 9]), ("dbg_J", [128, 4 * NJ * 3]),
                        ("dbg_ch", [128, 4 * 32 * 12]), ("dbg_d4", [128, BG * VS * 4]),
                        ("dbg_relT1", [128, 12 * 128]), ("dbg_pfh", [126, B])):
            tens[nm] = nc.dram_tensor(nm, shp, f32, kind="ExternalOutput")[:]
    tens["joints_out"] = nc.dram_tensor(
        "joints_out", [128, BG, NJ * 3], f32, kind="ExternalOutput")[:]

    with tile.TileContext(nc) as tc:
        _emit(nc, tc, tens)
    nc.compile()

    _CACHE[key] = (nc, tens)
    return nc, tens


def host_prep(betas, pose, v_template, shapedirs, posedirs, J_regressor,
              lbs_weights, parents):
    """Returns per-core input maps (numpy)."""
    import ml_dtypes

    f32 = np.float32
    cbdt = ml_dtypes.bfloat16 if CORR_DT == "bf16" else np.float32
    betas = np.asarray(betas, f32)
    pose = np.asarray(pose, f32)
    v_template = np.asarray(v_template, f32)
    shapedirs = np.asarray(shapedirs, f32)
    posedirs = np.asarray(posedirs, f32)
    J_regressor = np.asarray(J_regressor, f32)
    lbs_weights = np.asarray(lbs_weights, f32)

    pose_t = np.ascontiguousarray(
        pose.reshape(BG, 128, NJ, 3).transpose(1, 0, 2, 3))
    btaug = np.concatenate([betas.T, np.ones((1, B), f32)], axis=0)
    btaug = np.ascontiguousarray(btaug)
    jdirs_l = np.einsum("jv,vkl->ljk", J_regressor, shapedirs).reshape(NB, NJ * 3)
    jt = (J_regressor @ v_template).reshape(1, NJ * 3)
    jdirs = np.ascontiguousarray(np.concatenate([jdirs_l, jt], axis=0), f32)
    negI = np.zeros((207, 1), f32)
    for f in range(207):
        m, n = (f % 9) // 3, f % 3
        if m == n:
            negI[f, 0] = -1.0
    ident = np.eye(128, dtype=f32)

    # padded per-vertex params
    VP = NCORES * VS
    w_pad = np.zeros((VP, NJ), f32)
    w_pad[:V] = lbs_weights
    vt_pad = np.zeros((VP, 3), f32)
    vt_pad[:V] = v_template
    pd_pad = np.zeros((207, VP * 3), f32)
    pd_pad[:, :V * 3] = posedirs
    sd_pad = np.zeros((NB, VP * 3), f32)
    sd_pad[:, :V * 3] = shapedirs.transpose(2, 0, 1).reshape(NB, V * 3)

    in_maps = []
    for c in range(NCORES):
        vsl = slice(c * VS, (c + 1) * VS)
        csl = slice(c * VS * 3, (c + 1) * VS * 3)
        w_s = w_pad[vsl]                          # [VS, 24]
        vt_s = vt_pad[vsl]                        # [VS, 3]
        G = np.zeros((128, VS), f32)
        wT3 = np.zeros((88, VS), f32)
        for n in range(4):
            for j in range(NJ):
                G[n * 32 + j] = w_s[:, j] * (vt_s[:, n] if n < 3 else 1.0)
                if n < 3:
                    wT3[n * 32 + j] = w_s[:, j]
        in_maps.append({
            "pose_t": pose_t,
            "btaug": btaug,
            "jdirs": jdirs,
            "negI_hi": negI[:126],
            "negI_lo": negI[126:],
            "ident": ident,
            "G": G,
            "wT": wT3.astype(cbdt),
            "pd_hi": np.ascontiguousarray(pd_pad[:126, csl]).astype(cbdt),
            "pd_lo": np.ascontiguousarray(pd_pad[126:, csl]).astype(cbdt),
            "sdT": np.ascontiguousarray(sd_pad[:, csl]).astype(cbdt),
        })
    return in_maps


def assemble(results):
    """results: list of 8 dicts with verts_out/joints_out -> (verts, joints)."""
    verts = np.empty((B, V, 3), np.float32)
    for c in range(NCORES):
        vo = results[c]["verts_out"].reshape(3, BG, 128, VS)
        vo = vo.transpose(1, 2, 3, 0).reshape(B, VS, 3)
        n = min(VS, V - c * VS)
        verts[:, c * VS:c * VS + n] = vo[:, :n]
    jo = results[0]["joints_out"].reshape(128, BG, NJ, 3)
    joints = np.ascontiguousarray(jo.transpose(1, 0, 2, 3)).reshape(B, NJ, 3)
    return verts, joints


def kernel(**inputs):
    from concourse.bass_utils import run_bass_kernel_spmd

    nc, _ = _build()
    in_maps = host_prep(**inputs)
    res = run_bass_kernel_spmd(nc, in_maps, core_ids=list(range(NCORES)))
    return assemble(res.results)


def kernel_traced(**inputs):
    """Like kernel() but with NTFF profiling; returns (verts, joints, res)."""
    from concourse.bass_utils import run_bass_kernel_spmd

    nc, _ = _build()
    in_maps = host_prep(**inputs)
    res = run_bass_kernel_spmd(nc, in_maps, core_ids=list(range(NCORES)),
                               trace=True)
    verts, joints = assemble(res.results)
    return verts, joints, res


# revision 24
# speedup vs baseline: 1.0360x; 1.0360x over previous
"""SMPL body-model (B=512, V=6890, J=24) Bass kernel for 8 Trainium2 cores.

Strategy: vertex-shard V across the 8 cores (864 verts/core, zero-padded to
6912); every core computes the full batch B=512 for the small per-batch work
(Rodrigues, joint regression, kinematic chain) and its vertex slice for the
heavy per-vertex work.

Key algebraic restructure (avoids materializing [B,V,4,4] skinning mats):
  verts[b,v,m] = sum_{j,n} rel[b,j,m,n] * G[(n,j), v]              (term1)
               + sum_{j,n<3} rel[b,j,m,n] * w[v,j] * delta[b,v,n]  (corr)
  G[(n,j), v]  = w[v,j] * vt_h[n, v]   (batch-independent, host precomputed)
  delta[b,v,:] = blend-shape offsets + pose offsets (small magnitude)
so term1 is one K=96 fp32 matmul per output tile, and the correction runs
through K=24 matmuls (M3) + a per-vertex 4-wide multiply-reduce on DVE.
delta / M3 matmuls run as float32r (small-magnitude corrections only).
"""

import os
import numpy as np

B, V, NJ, NB = 512, 6890, 24, 10
NCORES = 8
VS = 864                    # vertex slice per core (8*864 = 6912 >= 6890)
VC = 432                    # vertex chunk (psum-bank aligned work unit)
BG = 4                      # batch groups of 128
PAR = [-1, 0, 0, 0, 1, 2, 3, 4, 5, 6, 7, 8, 9, 9, 9,
       12, 13, 14, 16, 17, 18, 19, 20, 21]

# matmul dtype for the correction path: "f32r" | "bf16" | "f32"
CORR_DT = os.environ.get("BODY_CORR_DT", "f32r")
STAGE = int(os.environ.get("BODY_STAGE", "99"))
DEBUG = bool(int(os.environ.get("BODY_DEBUG", "0")))
# dtype of the mul-reduce intermediate: bf16 (fast) or f32 (exact)
TMP_DT = os.environ.get("BODY_TMP_DT", "bf16")

_CACHE = {}


def _emit(nc, tc, tens):
    import concourse.bass as bass
    import concourse.mybir as mybir
    from concourse.bass import MemorySpace

    dt = mybir.dt
    f32 = dt.float32
    bf16 = dt.bfloat16
    AF = mybir.ActivationFunctionType
    ALU = mybir.AluOpType

    # dtype for correction-path matmul operand tiles; fp32r tiles must be
    # *produced* as fp32r (BIR verifier), so the tiles are declared f32r
    # and fp32 DRAM sources are bitcast at the DMA.
    corr_sb_dt = {"f32r": dt.float32r, "bf16": bf16, "f32": f32}[CORR_DT]

    def corr_src(ap):
        if CORR_DT == "f32r":
            return ap.bitcast(dt.float32r)
        return ap

    import contextlib
    stk = contextlib.ExitStack()
    cpool = stk.enter_context(tc.tile_pool(name="consts", bufs=1))
    wpool = stk.enter_context(tc.tile_pool(name="work", bufs=1))
    ppool = stk.enter_context(tc.tile_pool(name="ps", bufs=2, space="PSUM"))
    spool = stk.enter_context(tc.tile_pool(name="stream", bufs=3))

    # ---- load constants ----
    def load(name, shape, dtype=f32):
        t = cpool.tile(shape, dtype, tag=name)
        srcap = tens[name][:]
        if dtype == dt.float32r:
            srcap = srcap.bitcast(dt.float32r)
        nc.sync.dma_start(t[:], srcap)
        return t

    posef = load("pose_t", [128, BG, NJ, 3])
    bt = load("btaug", [NB + 1, B])
    btr = cpool.tile([NB, B], corr_sb_dt, tag="btr")
    nc.sync.dma_start(btr[:], corr_src(tens["btaug"][0:NB, :]))
    jd = load("jdirs", [NB + 1, NJ * 3])
    negIh = load("negI_hi", [126, 1])
    negIl = load("negI_lo", [81, 1])
    ident = load("ident", [128, 128])
    Ghi = load("G_hi", [128, VS], bf16)
    Glo = load("G_lo", [128, VS], bf16)
    wT = load("wT", [88, VS], corr_sb_dt)
    pdh = load("pd_hi", [126, VS * 3], corr_sb_dt)
    pdl = load("pd_lo", [81, VS * 3], corr_sb_dt)
    sdt = load("sdT", [NB, VS * 3], corr_sb_dt)

    verts_out = tens["verts_out"]
    joints_out = tens["joints_out"]

    if STAGE < 1:
        stk.close()
        return
    # ---- Rodrigues: pose [128,(bg,j,3)] -> R [128,(bg,j,3,3)] ----
    rv1 = wpool.tile([128, BG, NJ, 3], f32)
    nc.vector.tensor_scalar_add(rv1[:], posef[:], 1e-8)
    sq = wpool.tile([128, BG, NJ, 3], f32)
    nc.scalar.square(sq[:], rv1[:])
    th2 = wpool.tile([128, BG, NJ], f32)
    nc.vector.reduce_sum(th2[:], sq[:], axis=mybir.AxisListType.X)
    th = wpool.tile([128, BG, NJ], f32)
    nc.scalar.sqrt(th[:], th2[:])
    rth = wpool.tile([128, BG, NJ], f32)
    nc.vector.reciprocal(rth[:], th[:])
    sin = wpool.tile([128, BG, NJ], f32)
    nc.scalar.activation(sin[:], th[:], AF.Sin)
    thc = wpool.tile([128, BG, NJ], f32)
    nc.vector.tensor_scalar_add(thc[:], th[:], float(np.pi / 2))
    cos = wpool.tile([128, BG, NJ], f32)
    nc.scalar.activation(cos[:], thc[:], AF.Sin)
    omc = wpool.tile([128, BG, NJ], f32)
    nc.scalar.activation(omc[:], cos[:], AF.Identity, bias=1.0, scale=-1.0)

    if STAGE < 1.5 and STAGE >= 1:
        pass
    rd = wpool.tile([128, BG, NJ, 3], f32)
    nc.vector.tensor_mul(rd[:], posef[:],
                         rth[:].unsqueeze(3).broadcast_to([128, BG, NJ, 3]))
    srd = wpool.tile([128, BG, NJ, 3], f32)
    nc.vector.tensor_mul(srd[:], rd[:],
                         sin[:].unsqueeze(3).broadcast_to([128, BG, NJ, 3]))
    # R = omc * (rd rd^T);  then += cos on diag, += sin*K off-diag
    ord_t = wpool.tile([128, BG, NJ, 3], f32)
    nc.vector.tensor_mul(ord_t[:], rd[:],
                         omc[:].unsqueeze(3).broadcast_to([128, BG, NJ, 3]))
    R = wpool.tile([128, BG, NJ, 3, 3], f32)
    nc.vector.tensor_mul(
        R[:],
        rd[:].unsqueeze(4).broadcast_to([128, BG, NJ, 3, 3]),
        ord_t[:].unsqueeze(3).broadcast_to([128, BG, NJ, 3, 3]))
    Rf = R[:].rearrange("p g j m n -> p g j (m n)")
    diag = Rf[:, :, :, 0:9:4]
    nc.vector.tensor_add(diag, diag,
                         cos[:].unsqueeze(3).broadcast_to([128, BG, NJ, 3]))
    # K matrix: [[0,-z,y],[z,0,-x],[-y,x,0]] * sin
    for (mn, comp, sign) in ((1, 2, -1.0), (3, 2, 1.0), (2, 1, 1.0),
                             (6, 1, -1.0), (5, 0, -1.0), (7, 0, 1.0)):
        dst = Rf[:, :, :, mn:mn + 1].squeeze(3)
        src = srd[:, :, :, comp:comp + 1].squeeze(3)
        if sign > 0:
            nc.vector.tensor_add(dst, dst, src)
        else:
            nc.vector.tensor_sub(dst, dst, src)

    if STAGE < 2:
        stk.close()
        return
    if DEBUG:
        nc.sync.dma_start(tens["dbg_R"][:],
                          R[:].rearrange("p g j m n -> p (g j m n)"))

    # ---- pose-feature transposes: pfh [126, B], pfl [81, B] (rows (j,m,n)) ----
    pfh = wpool.tile([126, B], corr_sb_dt)
    pfl = wpool.tile([81, B], corr_sb_dt)
    pf_src = R[:].rearrange("p g j m n -> p g (j m n)")
    for bg in range(BG):
        for (lo, cnt, dstt, nI) in ((0, 126, pfh, negIh), (126, 81, pfl, negIl)):
            tp = ppool.tile([cnt, 128], f32, tag="ps")
            nc.tensor.transpose(
                tp[:], pf_src[:, bg, 9 + lo: 9 + lo + cnt], ident[:])
            nc.scalar.activation(
                dstt[:, bg * 128:(bg + 1) * 128], tp[:], AF.Identity,
                bias=nI[:, 0:1])

    if STAGE < 2:
        stk.close()
        return
    # ---- J = betas_aug @ jdirs_aug -> [128,(bg,j,3)] ----
    J = wpool.tile([128, BG, NJ, 3], f32)
    for bg in range(BG):
        jp = ppool.tile([128, NJ * 3], f32, tag="ps")
        nc.tensor.matmul(jp[:], bt[:, bg * 128:(bg + 1) * 128], jd[:])
        nc.scalar.copy(J[:, bg][:].rearrange("p j k -> p (j k)"), jp[:])

    if STAGE < 3:
        stk.close()
        return
    if DEBUG:
        nc.sync.dma_start(tens["dbg_pfh"][:], pfh[:])

    # ---- delta matmuls -> d4 [128,(bg,v,4)] bf16, 4th lane zero ----
    d4 = wpool.tile([128, BG, VS, 4], bf16)
    nc.gpsimd.memset(d4[:, :, :, 3:4], 0.0)
    for bg in range(BG):
        bsl = slice(bg * 128, (bg + 1) * 128)
        for vc in range(VS // VC):
            dp = ppool.tile([128, 3, 512], f32, tag="ps")
            for ch in range(3):
                csl = slice(vc * 3 * VC + ch * VC, vc * 3 * VC + (ch + 1) * VC)
                mms = ((btr, sdt), (pfh, pdh), (pfl, pdl))
                for i, (lhs, rhs) in enumerate(mms):
                    nc.tensor.matmul(
                        dp[:, ch, 0:VC], lhs[:, bsl], rhs[:, csl],
                        start=(i == 0), stop=(i == len(mms) - 1))
            # psum cols are (v,n)-interleaved: chunk ch = verts [ch*144,+144)
            nc.scalar.copy(
                d4[:, bg, vc * VC:(vc + 1) * VC, 0:3]
                .rearrange("p (c v) n -> p c v n", c=3),
                dp[:, :, 0:VC].rearrange("p c (v n) -> p c v n", n=3))

    if STAGE < 4:
        stk.close()
        return
    if DEBUG:
        nc.sync.dma_start(tens["dbg_J"][:], J[:].rearrange("p g j n -> p (g j n)"))
        nc.sync.dma_start(tens["dbg_d4"][:],
                          d4[:].rearrange("p g v n -> p (g v n)"))

    # ---- rel_joints (per kinematic-tree runs) ----
    rj = wpool.tile([128, BG, NJ, 3], f32)
    nc.vector.tensor_copy(rj[:, :, 0], J[:, :, 0])
    nc.vector.tensor_sub(rj[:, :, 1:4], J[:, :, 1:4],
                         J[:, :, 0:1].broadcast_to([128, BG, 3, 3]))
    nc.vector.tensor_sub(rj[:, :, 4:13], J[:, :, 4:13], J[:, :, 1:10])
    nc.vector.tensor_sub(rj[:, :, 13:15], J[:, :, 13:15],
                         J[:, :, 9:10].broadcast_to([128, BG, 2, 3]))
    nc.vector.tensor_sub(rj[:, :, 15:18], J[:, :, 15:18], J[:, :, 12:15])
    nc.vector.tensor_sub(rj[:, :, 18:24], J[:, :, 18:24], J[:, :, 16:22])

    if STAGE < 5:
        stk.close()
        return
    # ---- tmat: [R | rel_joints] 3x4 ----
    tm = wpool.tile([128, BG, NJ, 3, 4], f32)
    for bg in range(BG):
        nc.vector.tensor_copy(tm[:, bg, :, :, 0:3], R[:, bg])
    nc.vector.tensor_copy(tm[:, :, :, :, 3], rj[:])

    # ---- kinematic chain (3x4 affine composition) ----
    # j padded to 32 so the (n, j) transpose lands n-blocks at 32-aligned
    # partitions (matmul operand base must be 0/32/64); pad rows stay zero.
    ch_t = wpool.tile([128, BG, 32, 3, 4], f32)
    nc.gpsimd.memset(ch_t[:], 0.0)
    nc.vector.tensor_copy(ch_t[:, :, 0, :, :], tm[:, :, 0])
    tmpj = wpool.tile([128, BG, 3, 4, 3], f32)
    for j in range(1, NJ):
        p = PAR[j]
        for bg in range(BG):
            in0 = (ch_t[:, bg, p, :, 0:3].unsqueeze(2)
                   .broadcast_to([128, 3, 4, 3]))
            in1 = (tm[:, bg, j].unsqueeze(1).transpose([0, 1, 3, 2])
                   .broadcast_to([128, 3, 4, 3]))
            nc.vector.tensor_mul(tmpj[:, bg], in0, in1)
            nc.vector.reduce_sum(ch_t[:, bg, j, :, :], tmpj[:, bg],
                                 axis=mybir.AxisListType.X)
        nc.vector.tensor_add(ch_t[:, :, j, :, 3], ch_t[:, :, j, :, 3],
                             ch_t[:, :, p, :, 3])

    if STAGE < 6:
        stk.close()
        return
    # posed joints output (before the rel subtraction)
    jstage = wpool.tile([128, BG, NJ, 3], f32)
    nc.vector.tensor_copy(jstage[:], ch_t[:, :, 0:NJ, :, 3])
    nc.sync.dma_start(
        joints_out[:].rearrange("p g (j k) -> p g j k", k=3), jstage[:])

    if STAGE < 7:
        stk.close()
        return
    # ---- rel transforms: translation -= chain_rot @ J ----
    tj_t = wpool.tile([128, BG, NJ, 3, 3], f32)
    tj = wpool.tile([128, BG, NJ, 3], f32)
    for bg in range(BG):
        nc.vector.tensor_mul(
            tj_t[:, bg], ch_t[:, bg, 0:NJ, :, 0:3],
            J[:, bg].unsqueeze(2).broadcast_to([128, NJ, 3, 3]))
        nc.vector.reduce_sum(tj[:, bg], tj_t[:, bg],
                             axis=mybir.AxisListType.X)
    nc.vector.tensor_sub(ch_t[:, :, 0:NJ, :, 3], ch_t[:, :, 0:NJ, :, 3], tj[:])

    if STAGE < 8:
        stk.close()
        return
    if DEBUG:
        nc.sync.dma_start(tens["dbg_ch"][:],
                          ch_t[:].rearrange("p g j m n -> p (g j m n)"))

    # ---- transposes -> relT1 [128=(n,j32), 12=(m,bg), 128] ----
    relT1 = wpool.tile([128, 12, 128], f32)
    relT1b = (wpool.tile([128, 12, 128], corr_sb_dt, tag="relT1b",
                         name="relT1b")
              if CORR_DT != "f32" else None)
    relT1h = wpool.tile([128, 12, 128], bf16)
    relT1c = wpool.tile([128, 12, 128], bf16)
    for m in range(3):
        for bg in range(BG):
            # contiguous (n, j) staging: transpose moving op needs 1 free dim
            cstg = spool.tile([128, 128], f32, tag="cstg")
            nc.vector.tensor_copy(
                cstg[:].rearrange("p (n j) -> p n j", n=4),
                ch_t[:, bg, :, m, :].transpose([0, 2, 1]))
            tp = ppool.tile([128, 128], f32, tag="ps")
            nc.tensor.transpose(tp[:], cstg[:], ident[:])
            nc.scalar.copy(relT1[:, m * 4 + bg], tp[:])
            if relT1b is not None:
                nc.scalar.copy(relT1b[:, m * 4 + bg], tp[:])
            nc.scalar.copy(relT1h[:, m * 4 + bg], tp[:])
            nc.vector.tensor_sub(relT1c[:, m * 4 + bg], tp[:],
                                 relT1h[:, m * 4 + bg])

    if STAGE < 9:
        stk.close()
        return
    if DEBUG:
        nc.sync.dma_start(tens["dbg_relT1"][:],
                          relT1[:].rearrange("p c b -> p (c b)"))

    # ---- mv units: per (m, bg, vchunk): 3 M3 matmuls + term1 + mul-reduce ----
    relT3 = relT1b if relT1b is not None else relT1
    for m in range(3):
        for bg in range(BG):
            c = m * 4 + bg
            for vc in range(VS // VC):
                vsl = slice(vc * VC, (vc + 1) * VC)
                mv = ppool.tile([128, 4, 512], f32, tag="ps")
                for n in range(3):
                    nc.tensor.matmul(
                        mv[:, n, 0:VC],
                        relT3[n * 32:n * 32 + NJ, c],
                        wT[n * 32:n * 32 + NJ, vsl])
                # term1 in split-bf16: hi@Ghi + hi@Glo + lo@Ghi (fp32 psum)
                nc.tensor.matmul(mv[:, 3, 0:VC], relT1h[:, c], Ghi[:, vsl],
                                 start=True, stop=False)
                nc.tensor.matmul(mv[:, 3, 0:VC], relT1h[:, c], Glo[:, vsl],
                                 start=False, stop=False)
                nc.tensor.matmul(mv[:, 3, 0:VC], relT1c[:, c], Ghi[:, vsl],
                                 start=False, stop=True)
                # evict M3+T1 psum -> bf16 (v,n)-interleaved on ACT; lane 3
                # (T1) is multiplied by d4's zero lane so its bf16 loss is
                # irrelevant -- T1 is re-added in fp32 from psum below.
                mvs = spool.tile([128, VC, 4], bf16, tag="mvs")
                nc.scalar.copy(mvs[:], mv[:, :, 0:VC].transpose([0, 2, 1]))
                tmpb = spool.tile([128, VC, 4], bf16, tag="tmpb")
                nc.vector.tensor_mul(tmpb[:], mvs[:], d4[:, bg, vsl])
                vout = spool.tile([128, VC], f32, tag="vout")
                nc.vector.reduce_sum(vout[:], tmpb[:], axis=mybir.AxisListType.X)
                nc.vector.tensor_add(vout[:], vout[:], mv[:, 3, 0:VC])
                nc.sync.dma_start(verts_out[c, :, vsl], vout[:])

    stk.close()


def _build():
    key = ("nc", CORR_DT, TMP_DT)
    if key in _CACHE:
        return _CACHE[key]
    import concourse.bacc as bacc
    import concourse.tile as tile
    import concourse.mybir as mybir

    dt = mybir.dt
    f32 = dt.float32
    cbdt = dt.bfloat16 if CORR_DT == "bf16" else f32

    nc = bacc.Bacc("TRN2", target_bir_lowering=False, debug=False)
    tens = {}

    def din(name, shape, dtype=f32):
        tens[name] = nc.dram_tensor(name, shape, dtype, kind="ExternalInput")[:]

    din("pose_t", [128, BG, NJ, 3])
    din("btaug", [NB + 1, B])
    din("jdirs", [NB + 1, NJ * 3])
    din("negI_hi", [126, 1])
    din("negI_lo", [81, 1])
    din("ident", [128, 128])
    din("G_hi", [128, VS], dt.bfloat16)
    din("G_lo", [128, VS], dt.bfloat16)
    din("wT", [88, VS], cbdt)
    din("pd_hi", [126, VS * 3], cbdt)
    din("pd_lo", [81, VS * 3], cbdt)
    din("sdT", [NB, VS * 3], cbdt)
    tens["verts_out"] = nc.dram_tensor(
        "verts_out", [12, 128, VS], f32, kind="ExternalOutput")[:]
    if os.environ.get("BODY_DEBUG", "0") != "0":
        for nm, shp in (("dbg_R", [128, 4 * NJ * 9]), ("dbg_J", [128, 4 * NJ * 3]),
                        ("dbg_ch", [128, 4 * 32 * 12]), ("dbg_d4", [128, BG * VS * 4]),
                        ("dbg_relT1", [128, 12 * 128]), ("dbg_pfh", [126, B])):
            tens[nm] = nc.dram_tensor(nm, shp, f32, kind="ExternalOutput")[:]
    tens["joints_out"] = nc.dram_tensor(
        "joints_out", [128, BG, NJ * 3], f32, kind="ExternalOutput")[:]

    with tile.TileContext(nc) as tc:
        _emit(nc, tc, tens)
    nc.compile()

    _CACHE[key] = (nc, tens)
    return nc, tens


def host_prep(betas, pose, v_template, shapedirs, posedirs, J_regressor,
              lbs_weights, parents):
    """Returns per-core input maps (numpy)."""
    import ml_dtypes

    f32 = np.float32
    cbdt = ml_dtypes.bfloat16 if CORR_DT == "bf16" else np.float32
    betas = np.asarray(betas, f32)
    pose = np.asarray(pose, f32)
    v_template = np.asarray(v_template, f32)
    shapedirs = np.asarray(shapedirs, f32)
    posedirs = np.asarray(posedirs, f32)
    J_regressor = np.asarray(J_regressor, f32)
    lbs_weights = np.asarray(lbs_weights, f32)

    pose_t = np.ascontiguousarray(
        pose.reshape(BG, 128, NJ, 3).transpose(1, 0, 2, 3))
    btaug = np.concatenate([betas.T, np.ones((1, B), f32)], axis=0)
    btaug = np.ascontiguousarray(btaug)
    jdirs_l = np.einsum("jv,vkl->ljk", J_regressor, shapedirs).reshape(NB, NJ * 3)
    jt = (J_regressor @ v_template).reshape(1, NJ * 3)
    jdirs = np.ascontiguousarray(np.concatenate([jdirs_l, jt], axis=0), f32)
    negI = np.zeros((207, 1), f32)
    for f in range(207):
        m, n = (f % 9) // 3, f % 3
        if m == n:
            negI[f, 0] = -1.0
    ident = np.eye(128, dtype=f32)

    # padded per-vertex params
    VP = NCORES * VS
    w_pad = np.zeros((VP, NJ), f32)
    w_pad[:V] = lbs_weights
    vt_pad = np.zeros((VP, 3), f32)
    vt_pad[:V] = v_template
    pd_pad = np.zeros((207, VP * 3), f32)
    pd_pad[:, :V * 3] = posedirs
    sd_pad = np.zeros((NB, VP * 3), f32)
    sd_pad[:, :V * 3] = shapedirs.transpose(2, 0, 1).reshape(NB, V * 3)

    in_maps = []
    for c in range(NCORES):
        vsl = slice(c * VS, (c + 1) * VS)
        csl = slice(c * VS * 3, (c + 1) * VS * 3)
        w_s = w_pad[vsl]                          # [VS, 24]
        vt_s = vt_pad[vsl]                        # [VS, 3]
        G = np.zeros((128, VS), f32)
        wT3 = np.zeros((88, VS), f32)
        for n in range(4):
            for j in range(NJ):
                G[n * 32 + j] = w_s[:, j] * (vt_s[:, n] if n < 3 else 1.0)
                if n < 3:
                    wT3[n * 32 + j] = w_s[:, j]
        G_hi = G.astype(ml_dtypes.bfloat16)
        G_lo = (G - G_hi.astype(np.float32)).astype(ml_dtypes.bfloat16)
        in_maps.append({
            "pose_t": pose_t,
            "btaug": btaug,
            "jdirs": jdirs,
            "negI_hi": negI[:126],
            "negI_lo": negI[126:],
            "ident": ident,
            "G_hi": G_hi,
            "G_lo": G_lo,
            "wT": wT3.astype(cbdt),
            "pd_hi": np.ascontiguousarray(pd_pad[:126, csl]).astype(cbdt),
            "pd_lo": np.ascontiguousarray(pd_pad[126:, csl]).astype(cbdt),
            "sdT": np.ascontiguousarray(sd_pad[:, csl]).astype(cbdt),
        })
    return in_maps


def assemble(results):
    """results: list of 8 dicts with verts_out/joints_out -> (verts, joints)."""
    verts = np.empty((B, V, 3), np.float32)
    for c in range(NCORES):
        vo = results[c]["verts_out"].reshape(3, BG, 128, VS)
        vo = vo.transpose(1, 2, 3, 0).reshape(B, VS, 3)
        n = min(VS, V - c * VS)
        verts[:, c * VS:c * VS + n] = vo[:, :n]
    jo = results[0]["joints_out"].reshape(128, BG, NJ, 3)
    joints = np.ascontiguousarray(jo.transpose(1, 0, 2, 3)).reshape(B, NJ, 3)
    return verts, joints


def kernel(**inputs):
    from concourse.bass_utils import run_bass_kernel_spmd

    nc, _ = _build()
    in_maps = host_prep(**inputs)
    res = run_bass_kernel_spmd(nc, in_maps, core_ids=list(range(NCORES)))
    return assemble(res.results)


def kernel_traced(**inputs):
    """Like kernel() but with NTFF profiling; returns (verts, joints, res)."""
    from concourse.bass_utils import run_bass_kernel_spmd

    nc, _ = _build()
    in_maps = host_prep(**inputs)
    res = run_bass_kernel_spmd(nc, in_maps, core_ids=list(range(NCORES)),
                               trace=True)
    verts, joints = assemble(res.results)
    return verts, joints, res


# revision 25
# speedup vs baseline: 1.0683x; 1.0312x over previous
"""SMPL body-model (B=512, V=6890, J=24) Bass kernel for 8 Trainium2 cores.

Strategy: vertex-shard V across the 8 cores (864 verts/core, zero-padded to
6912); every core computes the full batch B=512 for the small per-batch work
(Rodrigues, joint regression, kinematic chain) and its vertex slice for the
heavy per-vertex work.

Key algebraic restructure (avoids materializing [B,V,4,4] skinning mats):
  verts[b,v,m] = sum_{j,n} rel[b,j,m,n] * G[(n,j), v]              (term1)
               + sum_{j,n<3} rel[b,j,m,n] * w[v,j] * delta[b,v,n]  (corr)
  G[(n,j), v]  = w[v,j] * vt_h[n, v]   (batch-independent, host precomputed)
  delta[b,v,:] = blend-shape offsets + pose offsets (small magnitude)
so term1 is one K=96 fp32 matmul per output tile, and the correction runs
through K=24 matmuls (M3) + a per-vertex 4-wide multiply-reduce on DVE.
delta / M3 matmuls run as float32r (small-magnitude corrections only).
"""

import os
import numpy as np

B, V, NJ, NB = 512, 6890, 24, 10
NCORES = 8
VS = 864                    # vertex slice per core (8*864 = 6912 >= 6890)
VC = 432                    # vertex chunk (psum-bank aligned work unit)
BG = 4                      # batch groups of 128
PAR = [-1, 0, 0, 0, 1, 2, 3, 4, 5, 6, 7, 8, 9, 9, 9,
       12, 13, 14, 16, 17, 18, 19, 20, 21]

# matmul dtype for the correction path: "f32r" | "bf16" | "f32"
CORR_DT = os.environ.get("BODY_CORR_DT", "f32r")
STAGE = int(os.environ.get("BODY_STAGE", "99"))
DEBUG = bool(int(os.environ.get("BODY_DEBUG", "0")))
# dtype of the mul-reduce intermediate: bf16 (fast) or f32 (exact)
TMP_DT = os.environ.get("BODY_TMP_DT", "bf16")

_CACHE = {}


def _emit(nc, tc, tens):
    import concourse.bass as bass
    import concourse.mybir as mybir
    from concourse.bass import MemorySpace

    dt = mybir.dt
    f32 = dt.float32
    bf16 = dt.bfloat16
    AF = mybir.ActivationFunctionType
    ALU = mybir.AluOpType

    # dtype for correction-path matmul operand tiles; fp32r tiles must be
    # *produced* as fp32r (BIR verifier), so the tiles are declared f32r
    # and fp32 DRAM sources are bitcast at the DMA.
    corr_sb_dt = {"f32r": dt.float32r, "bf16": bf16, "f32": f32}[CORR_DT]

    def corr_src(ap):
        if CORR_DT == "f32r":
            return ap.bitcast(dt.float32r)
        return ap

    import contextlib
    stk = contextlib.ExitStack()
    cpool = stk.enter_context(tc.tile_pool(name="consts", bufs=1))
    wpool = stk.enter_context(tc.tile_pool(name="work", bufs=1))
    ppool = stk.enter_context(tc.tile_pool(name="ps", bufs=2, space="PSUM"))
    spool = stk.enter_context(tc.tile_pool(name="stream", bufs=3))

    # ---- load constants ----
    def load(name, shape, dtype=f32):
        t = cpool.tile(shape, dtype, tag=name)
        srcap = tens[name][:]
        if dtype == dt.float32r:
            srcap = srcap.bitcast(dt.float32r)
        nc.sync.dma_start(t[:], srcap)
        return t

    posef = load("pose_t", [128, BG, NJ, 3])
    bt = load("btaug", [NB + 1, B])
    btr = cpool.tile([NB, B], corr_sb_dt, tag="btr")
    nc.sync.dma_start(btr[:], corr_src(tens["btaug"][0:NB, :]))
    jd = load("jdirs", [NB + 1, NJ * 3])
    negIh = load("negI_hi", [126, 1])
    negIl = load("negI_lo", [81, 1])
    ident = load("ident", [128, 128])
    Ghi = load("G_hi", [128, VS], bf16)
    Glo = load("G_lo", [128, VS], bf16)
    wT = load("wT", [88, VS], corr_sb_dt)
    pdh = load("pd_hi", [126, VS * 3], corr_sb_dt)
    pdl = load("pd_lo", [81, VS * 3], corr_sb_dt)
    sdt = load("sdT", [NB, VS * 3], corr_sb_dt)

    verts_out = tens["verts_out"]
    joints_out = tens["joints_out"]

    if STAGE < 1:
        stk.close()
        return
    # ---- Rodrigues: pose [128,(bg,j,3)] -> R [128,(bg,j,3,3)] ----
    rv1 = wpool.tile([128, BG, NJ, 3], f32)
    nc.scalar.add(rv1[:], posef[:], 1e-8)
    sq = wpool.tile([128, BG, NJ, 3], f32)
    nc.scalar.square(sq[:], rv1[:])
    th2 = wpool.tile([128, BG, NJ], f32)
    nc.vector.reduce_sum(th2[:], sq[:], axis=mybir.AxisListType.X)
    th = wpool.tile([128, BG, NJ], f32)
    nc.scalar.sqrt(th[:], th2[:])
    rth = wpool.tile([128, BG, NJ], f32)
    nc.vector.reciprocal(rth[:], th[:])
    sin = wpool.tile([128, BG, NJ], f32)
    nc.scalar.activation(sin[:], th[:], AF.Sin)
    cos = wpool.tile([128, BG, NJ], f32)
    nc.scalar.activation(cos[:], th[:], AF.Sin, bias=float(np.pi / 2))
    omc = wpool.tile([128, BG, NJ], f32)
    nc.scalar.activation(omc[:], cos[:], AF.Identity, bias=1.0, scale=-1.0)

    if STAGE < 1.5 and STAGE >= 1:
        pass
    rd = wpool.tile([128, BG, NJ, 3], f32)
    nc.vector.tensor_mul(rd[:], posef[:],
                         rth[:].unsqueeze(3).broadcast_to([128, BG, NJ, 3]))
    srd = wpool.tile([128, BG, NJ, 3], f32)
    nc.vector.tensor_mul(srd[:], rd[:],
                         sin[:].unsqueeze(3).broadcast_to([128, BG, NJ, 3]))
    # R = omc * (rd rd^T);  then += cos on diag, += sin*K off-diag
    ord_t = wpool.tile([128, BG, NJ, 3], f32)
    nc.vector.tensor_mul(ord_t[:], rd[:],
                         omc[:].unsqueeze(3).broadcast_to([128, BG, NJ, 3]))
    R = wpool.tile([128, BG, NJ, 3, 3], f32)
    nc.gpsimd.tensor_mul(
        R[:],
        rd[:].unsqueeze(4).broadcast_to([128, BG, NJ, 3, 3]),
        ord_t[:].unsqueeze(3).broadcast_to([128, BG, NJ, 3, 3]))
    Rf = R[:].rearrange("p g j m n -> p g j (m n)")
    diag = Rf[:, :, :, 0:9:4]
    nc.vector.tensor_add(diag, diag,
                         cos[:].unsqueeze(3).broadcast_to([128, BG, NJ, 3]))
    # K matrix: [[0,-z,y],[z,0,-x],[-y,x,0]] * sin
    for (mn, comp, sign) in ((1, 2, -1.0), (3, 2, 1.0), (2, 1, 1.0),
                             (6, 1, -1.0), (5, 0, -1.0), (7, 0, 1.0)):
        dst = Rf[:, :, :, mn:mn + 1].squeeze(3)
        src = srd[:, :, :, comp:comp + 1].squeeze(3)
        if sign > 0:
            nc.vector.tensor_add(dst, dst, src)
        else:
            nc.vector.tensor_sub(dst, dst, src)

    if STAGE < 2:
        stk.close()
        return
    if DEBUG:
        nc.sync.dma_start(tens["dbg_R"][:],
                          R[:].rearrange("p g j m n -> p (g j m n)"))

    # ---- pose-feature transposes: pfh [126, B], pfl [81, B] (rows (j,m,n)) ----
    pfh = wpool.tile([126, B], corr_sb_dt)
    pfl = wpool.tile([81, B], corr_sb_dt)
    pf_src = R[:].rearrange("p g j m n -> p g (j m n)")
    for bg in range(BG):
        for (lo, cnt, dstt, nI) in ((0, 126, pfh, negIh), (126, 81, pfl, negIl)):
            tp = ppool.tile([cnt, 128], f32, tag="ps")
            nc.tensor.transpose(
                tp[:], pf_src[:, bg, 9 + lo: 9 + lo + cnt], ident[:])
            nc.scalar.activation(
                dstt[:, bg * 128:(bg + 1) * 128], tp[:], AF.Identity,
                bias=nI[:, 0:1])

    if STAGE < 2:
        stk.close()
        return
    # ---- J = betas_aug @ jdirs_aug -> [128,(bg,j,3)] ----
    J = wpool.tile([128, BG, NJ, 3], f32)
    for bg in range(BG):
        jp = ppool.tile([128, NJ * 3], f32, tag="ps")
        nc.tensor.matmul(jp[:], bt[:, bg * 128:(bg + 1) * 128], jd[:])
        nc.scalar.copy(J[:, bg][:].rearrange("p j k -> p (j k)"), jp[:])

    if STAGE < 3:
        stk.close()
        return
    if DEBUG:
        nc.sync.dma_start(tens["dbg_pfh"][:], pfh[:])

    # ---- delta matmuls -> d4 [128,(bg,n,v)] bf16 plane-major ----
    d4 = wpool.tile([128, BG, 3, VS], bf16)
    for bg in range(BG):
        bsl = slice(bg * 128, (bg + 1) * 128)
        for vc in range(VS // VC):
            dp = ppool.tile([128, 3, 512], f32, tag="ps")
            for ch in range(3):
                csl = slice(vc * 3 * VC + ch * VC, vc * 3 * VC + (ch + 1) * VC)
                mms = ((btr, sdt), (pfh, pdh), (pfl, pdl))
                for i, (lhs, rhs) in enumerate(mms):
                    nc.tensor.matmul(
                        dp[:, ch, 0:VC], lhs[:, bsl], rhs[:, csl],
                        start=(i == 0), stop=(i == len(mms) - 1))
            # psum cols are (v,n)-interleaved: chunk ch = verts [ch*144,+144)
            nc.scalar.copy(
                d4[:, bg, :, vc * VC:(vc + 1) * VC]
                .rearrange("p n (c v) -> p c n v", c=3),
                dp[:, :, 0:VC].rearrange("p c (v n) -> p c n v", n=3))

    if STAGE < 4:
        stk.close()
        return
    if DEBUG:
        nc.sync.dma_start(tens["dbg_J"][:], J[:].rearrange("p g j n -> p (g j n)"))
        nc.sync.dma_start(tens["dbg_d4"][:],
                          d4[:].rearrange("p g v n -> p (g v n)"))

    # ---- rel_joints (per kinematic-tree runs) ----
    rj = wpool.tile([128, BG, NJ, 3], f32)
    nc.gpsimd.tensor_copy(rj[:, :, 0], J[:, :, 0])
    nc.gpsimd.tensor_sub(rj[:, :, 1:4], J[:, :, 1:4],
                         J[:, :, 0:1].broadcast_to([128, BG, 3, 3]))
    nc.gpsimd.tensor_sub(rj[:, :, 4:13], J[:, :, 4:13], J[:, :, 1:10])
    nc.gpsimd.tensor_sub(rj[:, :, 13:15], J[:, :, 13:15],
                         J[:, :, 9:10].broadcast_to([128, BG, 2, 3]))
    nc.gpsimd.tensor_sub(rj[:, :, 15:18], J[:, :, 15:18], J[:, :, 12:15])
    nc.gpsimd.tensor_sub(rj[:, :, 18:24], J[:, :, 18:24], J[:, :, 16:22])

    if STAGE < 5:
        stk.close()
        return
    # ---- tmat: [R | rel_joints] 3x4 ----
    tm = wpool.tile([128, BG, NJ, 3, 4], f32)
    for bg in range(BG):
        nc.gpsimd.tensor_copy(tm[:, bg, :, :, 0:3], R[:, bg])
    nc.gpsimd.tensor_copy(tm[:, :, :, :, 3], rj[:])

    # ---- kinematic chain (3x4 affine composition) ----
    # j padded to 32 so the (n, j) transpose lands n-blocks at 32-aligned
    # partitions (matmul operand base must be 0/32/64); pad rows stay zero.
    ch_t = wpool.tile([128, BG, 32, 3, 4], f32)
    nc.gpsimd.memset(ch_t[:], 0.0)
    nc.gpsimd.tensor_copy(ch_t[:, :, 0, :, :], tm[:, :, 0])
    tmpj = wpool.tile([128, BG, 3, 4, 3], f32)
    for bg in range(BG):
        for j in range(1, NJ):
            p = PAR[j]
            in0 = (ch_t[:, bg, p, :, 0:3].unsqueeze(2)
                   .broadcast_to([128, 3, 4, 3]))
            in1 = (tm[:, bg, j].unsqueeze(1).transpose([0, 1, 3, 2])
                   .broadcast_to([128, 3, 4, 3]))
            nc.gpsimd.tensor_mul(tmpj[:, bg], in0, in1)
            nc.vector.reduce_sum(ch_t[:, bg, j, :, :], tmpj[:, bg],
                                 axis=mybir.AxisListType.X)
            nc.vector.tensor_add(ch_t[:, bg, j, :, 3], ch_t[:, bg, j, :, 3],
                                 ch_t[:, bg, p, :, 3])

    if STAGE < 6:
        stk.close()
        return
    # posed joints output (before the rel subtraction)
    jstage = wpool.tile([128, BG, NJ, 3], f32)
    nc.vector.tensor_copy(jstage[:], ch_t[:, :, 0:NJ, :, 3])
    nc.sync.dma_start(
        joints_out[:].rearrange("p g (j k) -> p g j k", k=3), jstage[:])

    if STAGE < 7:
        stk.close()
        return
    # ---- rel transforms: translation -= chain_rot @ J ----
    tj_t = wpool.tile([128, BG, NJ, 3, 3], f32)
    tj = wpool.tile([128, BG, NJ, 3], f32)
    for bg in range(BG):
        nc.vector.tensor_mul(
            tj_t[:, bg], ch_t[:, bg, 0:NJ, :, 0:3],
            J[:, bg].unsqueeze(2).broadcast_to([128, NJ, 3, 3]))
        nc.vector.reduce_sum(tj[:, bg], tj_t[:, bg],
                             axis=mybir.AxisListType.X)
    nc.vector.tensor_sub(ch_t[:, :, 0:NJ, :, 3], ch_t[:, :, 0:NJ, :, 3], tj[:])

    if STAGE < 8:
        stk.close()
        return
    if DEBUG:
        nc.sync.dma_start(tens["dbg_ch"][:],
                          ch_t[:].rearrange("p g j m n -> p (g j m n)"))

    # ---- transposes -> relT1 [128=(n,j32), 12=(m,bg), 128] ----
    relT1 = wpool.tile([128, 12, 128], f32)
    relT1b = (wpool.tile([128, 12, 128], corr_sb_dt, tag="relT1b",
                         name="relT1b")
              if CORR_DT != "f32" else None)
    relT1h = wpool.tile([128, 12, 128], bf16)
    relT1c = wpool.tile([128, 12, 128], bf16)
    for bg in range(BG):
        for m in range(3):
            # contiguous (n, j) staging: transpose moving op needs 1 free dim
            cstg = spool.tile([128, 128], f32, tag="cstg")
            nc.gpsimd.tensor_copy(
                cstg[:].rearrange("p (n j) -> p n j", n=4),
                ch_t[:, bg, :, m, :].transpose([0, 2, 1]))
            tp = ppool.tile([128, 128], f32, tag="ps")
            nc.tensor.transpose(tp[:], cstg[:], ident[:])
            nc.scalar.copy(relT1[:, m * 4 + bg], tp[:])
            if relT1b is not None:
                nc.scalar.copy(relT1b[:, m * 4 + bg], tp[:])
            nc.scalar.copy(relT1h[:, m * 4 + bg], tp[:])
            nc.vector.tensor_sub(relT1c[:, m * 4 + bg], tp[:],
                                 relT1h[:, m * 4 + bg])

    if STAGE < 9:
        stk.close()
        return
    if DEBUG:
        nc.sync.dma_start(tens["dbg_relT1"][:],
                          relT1[:].rearrange("p c b -> p (c b)"))

    # ---- mv units: per (m, bg, vchunk): 3 M3 matmuls + term1 + mul-reduce ----
    relT3 = relT1b if relT1b is not None else relT1
    for bg in range(BG):
        for m in range(3):
            c = m * 4 + bg
            for vc in range(VS // VC):
                vsl = slice(vc * VC, (vc + 1) * VC)
                mv = ppool.tile([128, 4, 512], f32, tag="ps")
                for n in range(3):
                    nc.tensor.matmul(
                        mv[:, n, 0:VC],
                        relT3[n * 32:n * 32 + NJ, c],
                        wT[n * 32:n * 32 + NJ, vsl])
                # term1 in split-bf16: hi@Ghi + hi@Glo + lo@Ghi (fp32 psum)
                nc.tensor.matmul(mv[:, 3, 0:VC], relT1h[:, c], Ghi[:, vsl],
                                 start=True, stop=False)
                nc.tensor.matmul(mv[:, 3, 0:VC], relT1h[:, c], Glo[:, vsl],
                                 start=False, stop=False)
                nc.tensor.matmul(mv[:, 3, 0:VC], relT1c[:, c], Ghi[:, vsl],
                                 start=False, stop=True)
                # evict the 3 M3 planes to bf16 (plane-major) on ACT
                mvs = spool.tile([128, 3, VC], bf16, tag="mvs")
                nc.scalar.copy(mvs[:], mv[:, 0:3, 0:VC])
                # corr = sum_n plane_n * delta_n  (bf16 2x TT adds)
                tmpb = spool.tile([128, 3, VC], bf16, tag="tmpb")
                nc.vector.tensor_mul(tmpb[:], mvs[:], d4[:, bg, :, vsl])
                s01 = spool.tile([128, VC], bf16, tag="s01")
                nc.gpsimd.tensor_add(s01[:], tmpb[:, 0], tmpb[:, 1])
                s012 = spool.tile([128, VC], f32, tag="s012")
                nc.vector.tensor_add(s012[:], s01[:], tmpb[:, 2])
                vout = spool.tile([128, VC], f32, tag="vout")
                nc.vector.tensor_add(vout[:], s012[:], mv[:, 3, 0:VC])
                nc.sync.dma_start(verts_out[c, :, vsl], vout[:])

    stk.close()


def _build():
    key = ("nc", CORR_DT, TMP_DT)
    if key in _CACHE:
        return _CACHE[key]
    import concourse.bacc as bacc
    import concourse.tile as tile
    import concourse.mybir as mybir

    dt = mybir.dt
    f32 = dt.float32
    cbdt = dt.bfloat16 if CORR_DT == "bf16" else f32

    nc = bacc.Bacc("TRN2", target_bir_lowering=False, debug=False)
    # extra const APs for ACT biases (preamble-style: memset + barrier)
    import numpy as _np
    for val in (1e-8, float(_np.pi / 2)):
        t = nc.alloc_sbuf_tensor(f"constx-{val}", [128, 1], dt.float32)
        nc.gpsimd.memset(t.ap(), val)
        nc.const_aps.aps[(dt.float32, val)] = t.ap()
    nc.all_engine_barrier()
    tens = {}

    def din(name, shape, dtype=f32):
        tens[name] = nc.dram_tensor(name, shape, dtype, kind="ExternalInput")[:]

    din("pose_t", [128, BG, NJ, 3])
    din("btaug", [NB + 1, B])
    din("jdirs", [NB + 1, NJ * 3])
    din("negI_hi", [126, 1])
    din("negI_lo", [81, 1])
    din("ident", [128, 128])
    din("G_hi", [128, VS], dt.bfloat16)
    din("G_lo", [128, VS], dt.bfloat16)
    din("wT", [88, VS], cbdt)
    din("pd_hi", [126, VS * 3], cbdt)
    din("pd_lo", [81, VS * 3], cbdt)
    din("sdT", [NB, VS * 3], cbdt)
    tens["verts_out"] = nc.dram_tensor(
        "verts_out", [12, 128, VS], f32, kind="ExternalOutput")[:]
    if os.environ.get("BODY_DEBUG", "0") != "0":
        for nm, shp in (("dbg_R", [128, 4 * NJ * 9]), ("dbg_J", [128, 4 * NJ * 3]),
                        ("dbg_ch", [128, 4 * 32 * 12]), ("dbg_d4", [128, BG * VS * 4]),
                        ("dbg_relT1", [128, 12 * 128]), ("dbg_pfh", [126, B])):
            tens[nm] = nc.dram_tensor(nm, shp, f32, kind="ExternalOutput")[:]
    tens["joints_out"] = nc.dram_tensor(
        "joints_out", [128, BG, NJ * 3], f32, kind="ExternalOutput")[:]

    with tile.TileContext(nc) as tc:
        _emit(nc, tc, tens)
    nc.compile()

    _CACHE[key] = (nc, tens)
    return nc, tens


def host_prep(betas, pose, v_template, shapedirs, posedirs, J_regressor,
              lbs_weights, parents):
    """Returns per-core input maps (numpy)."""
    import ml_dtypes

    f32 = np.float32
    cbdt = ml_dtypes.bfloat16 if CORR_DT == "bf16" else np.float32
    betas = np.asarray(betas, f32)
    pose = np.asarray(pose, f32)
    v_template = np.asarray(v_template, f32)
    shapedirs = np.asarray(shapedirs, f32)
    posedirs = np.asarray(posedirs, f32)
    J_regressor = np.asarray(J_regressor, f32)
    lbs_weights = np.asarray(lbs_weights, f32)

    pose_t = np.ascontiguousarray(
        pose.reshape(BG, 128, NJ, 3).transpose(1, 0, 2, 3))
    btaug = np.concatenate([betas.T, np.ones((1, B), f32)], axis=0)
    btaug = np.ascontiguousarray(btaug)
    jdirs_l = np.einsum("jv,vkl->ljk", J_regressor, shapedirs).reshape(NB, NJ * 3)
    jt = (J_regressor @ v_template).reshape(1, NJ * 3)
    jdirs = np.ascontiguousarray(np.concatenate([jdirs_l, jt], axis=0), f32)
    negI = np.zeros((207, 1), f32)
    for f in range(207):
        m, n = (f % 9) // 3, f % 3
        if m == n:
            negI[f, 0] = -1.0
    ident = np.eye(128, dtype=f32)

    # padded per-vertex params
    VP = NCORES * VS
    w_pad = np.zeros((VP, NJ), f32)
    w_pad[:V] = lbs_weights
    vt_pad = np.zeros((VP, 3), f32)
    vt_pad[:V] = v_template
    pd_pad = np.zeros((207, VP * 3), f32)
    pd_pad[:, :V * 3] = posedirs
    sd_pad = np.zeros((NB, VP * 3), f32)
    sd_pad[:, :V * 3] = shapedirs.transpose(2, 0, 1).reshape(NB, V * 3)

    in_maps = []
    for c in range(NCORES):
        vsl = slice(c * VS, (c + 1) * VS)
        csl = slice(c * VS * 3, (c + 1) * VS * 3)
        w_s = w_pad[vsl]                          # [VS, 24]
        vt_s = vt_pad[vsl]                        # [VS, 3]
        G = np.zeros((128, VS), f32)
        wT3 = np.zeros((88, VS), f32)
        for n in range(4):
            for j in range(NJ):
                G[n * 32 + j] = w_s[:, j] * (vt_s[:, n] if n < 3 else 1.0)
                if n < 3:
                    wT3[n * 32 + j] = w_s[:, j]
        G_hi = G.astype(ml_dtypes.bfloat16)
        G_lo = (G - G_hi.astype(np.float32)).astype(ml_dtypes.bfloat16)
        in_maps.append({
            "pose_t": pose_t,
            "btaug": btaug,
            "jdirs": jdirs,
            "negI_hi": negI[:126],
            "negI_lo": negI[126:],
            "ident": ident,
            "G_hi": G_hi,
            "G_lo": G_lo,
            "wT": wT3.astype(cbdt),
            "pd_hi": np.ascontiguousarray(pd_pad[:126, csl]).astype(cbdt),
            "pd_lo": np.ascontiguousarray(pd_pad[126:, csl]).astype(cbdt),
            "sdT": np.ascontiguousarray(sd_pad[:, csl]).astype(cbdt),
        })
    return in_maps


def assemble(results):
    """results: list of 8 dicts with verts_out/joints_out -> (verts, joints)."""
    verts = np.empty((B, V, 3), np.float32)
    for c in range(NCORES):
        vo = results[c]["verts_out"].reshape(3, BG, 128, VS)
        vo = vo.transpose(1, 2, 3, 0).reshape(B, VS, 3)
        n = min(VS, V - c * VS)
        verts[:, c * VS:c * VS + n] = vo[:, :n]
    jo = results[0]["joints_out"].reshape(128, BG, NJ, 3)
    joints = np.ascontiguousarray(jo.transpose(1, 0, 2, 3)).reshape(B, NJ, 3)
    return verts, joints


def kernel(**inputs):
    from concourse.bass_utils import run_bass_kernel_spmd

    nc, _ = _build()
    in_maps = host_prep(**inputs)
    res = run_bass_kernel_spmd(nc, in_maps, core_ids=list(range(NCORES)))
    return assemble(res.results)


def kernel_traced(**inputs):
    """Like kernel() but with NTFF profiling; returns (verts, joints, res)."""
    from concourse.bass_utils import run_bass_kernel_spmd

    nc, _ = _build()
    in_maps = host_prep(**inputs)
    res = run_bass_kernel_spmd(nc, in_maps, core_ids=list(range(NCORES)),
                               trace=True)
    verts, joints = assemble(res.results)
    return verts, joints, res


# revision 28
# speedup vs baseline: 1.3685x; 1.2811x over previous
"""SMPL body-model (B=512, V=6890, J=24) Bass kernel for 8 Trainium2 cores.

Strategy: vertex-shard V across the 8 cores (864 verts/core, zero-padded to
6912); every core computes the full batch B=512 for the small per-batch work
(Rodrigues, joint regression, kinematic chain) and its vertex slice for the
heavy per-vertex work.

Key algebraic restructure (avoids materializing [B,V,4,4] skinning mats):
  verts[b,v,m] = sum_{j,n} rel[b,j,m,n] * G[(n,j), v]              (term1)
               + sum_{j,n<3} rel[b,j,m,n] * w[v,j] * delta[b,v,n]  (corr)
  G[(n,j), v]  = w[v,j] * vt_h[n, v]   (batch-independent, host precomputed)
  delta[b,v,:] = blend-shape offsets + pose offsets (small magnitude)
so term1 is one K=96 fp32 matmul per output tile, and the correction runs
through K=24 matmuls (M3) + a per-vertex 4-wide multiply-reduce on DVE.
delta / M3 matmuls run as float32r (small-magnitude corrections only).
"""

import os
import numpy as np

B, V, NJ, NB = 512, 6890, 24, 10
NCORES = 8
VS = 864                    # vertex slice per core (8*864 = 6912 >= 6890)
VC = 432                    # vertex chunk (psum-bank aligned work unit)
BG = 4                      # batch groups of 128
PAR = [-1, 0, 0, 0, 1, 2, 3, 4, 5, 6, 7, 8, 9, 9, 9,
       12, 13, 14, 16, 17, 18, 19, 20, 21]

# matmul dtype for the correction path: "f32r" | "bf16" | "f32"
CORR_DT = os.environ.get("BODY_CORR_DT", "f32r")
STAGE = int(os.environ.get("BODY_STAGE", "99"))
DEBUG = bool(int(os.environ.get("BODY_DEBUG", "0")))
# dtype of the mul-reduce intermediate: bf16 (fast) or f32 (exact)
TMP_DT = os.environ.get("BODY_TMP_DT", "bf16")

_CACHE = {}


def _emit(nc, tc, tens):
    import concourse.bass as bass
    import concourse.mybir as mybir
    from concourse.bass import MemorySpace

    dt = mybir.dt
    f32 = dt.float32
    bf16 = dt.bfloat16
    AF = mybir.ActivationFunctionType
    ALU = mybir.AluOpType

    # dtype for correction-path matmul operand tiles; fp32r tiles must be
    # *produced* as fp32r (BIR verifier), so the tiles are declared f32r
    # and fp32 DRAM sources are bitcast at the DMA.
    corr_sb_dt = {"f32r": dt.float32r, "bf16": bf16, "f32": f32}[CORR_DT]

    def corr_src(ap):
        if CORR_DT == "f32r":
            return ap.bitcast(dt.float32r)
        return ap

    import contextlib
    stk = contextlib.ExitStack()
    cpool = stk.enter_context(tc.tile_pool(name="consts", bufs=1))
    wpool = stk.enter_context(tc.tile_pool(name="work", bufs=1))
    ppool = stk.enter_context(tc.tile_pool(name="ps", bufs=2, space="PSUM"))
    spool = stk.enter_context(tc.tile_pool(name="stream", bufs=3))

    # ---- load constants ----
    def load(name, shape, dtype=f32):
        t = cpool.tile(shape, dtype, tag=name)
        srcap = tens[name][:]
        if dtype == dt.float32r:
            srcap = srcap.bitcast(dt.float32r)
        nc.sync.dma_start(t[:], srcap)
        return t

    posef = load("pose_t", [128, BG, NJ, 3])
    bt = load("btaug", [NB + 1, B])
    btr = cpool.tile([NB, B], corr_sb_dt, tag="btr")
    nc.sync.dma_start(btr[:], corr_src(tens["btaug"][0:NB, :]))
    jd = load("jdirs", [NB + 1, NJ * 3])
    negIh = load("negI_hi", [126, 1])
    negIl = load("negI_lo", [81, 1])
    ident = load("ident", [128, 128])
    G = load("G", [128, VS])
    wT = load("wT", [88, VS], corr_sb_dt)
    pdh = load("pd_hi", [126, VS * 3], corr_sb_dt)
    pdl = load("pd_lo", [81, VS * 3], corr_sb_dt)
    sdt = load("sdT", [NB, VS * 3], corr_sb_dt)

    verts_out = tens["verts_out"]
    joints_out = tens["joints_out"]

    if STAGE < 1:
        stk.close()
        return
    # ---- Rodrigues: pose [128,(bg,j,3)] -> R [128,(bg,j,3,3)] ----
    rv1 = wpool.tile([128, BG, NJ, 3], f32)
    nc.scalar.add(rv1[:], posef[:], 1e-8)
    sq = wpool.tile([128, BG, NJ, 3], f32)
    nc.scalar.square(sq[:], rv1[:])
    th2 = wpool.tile([128, BG, NJ], f32)
    nc.vector.reduce_sum(th2[:], sq[:], axis=mybir.AxisListType.X)
    th = wpool.tile([128, BG, NJ], f32)
    nc.scalar.sqrt(th[:], th2[:])
    rth = wpool.tile([128, BG, NJ], f32)
    nc.vector.reciprocal(rth[:], th[:])
    sin = wpool.tile([128, BG, NJ], f32)
    nc.scalar.activation(sin[:], th[:], AF.Sin)
    cos = wpool.tile([128, BG, NJ], f32)
    nc.scalar.activation(cos[:], th[:], AF.Sin, bias=float(np.pi / 2))
    omc = wpool.tile([128, BG, NJ], f32)
    nc.scalar.activation(omc[:], cos[:], AF.Identity, bias=1.0, scale=-1.0)

    if STAGE < 1.5 and STAGE >= 1:
        pass
    rd = wpool.tile([128, BG, NJ, 3], f32)
    nc.vector.tensor_mul(rd[:], posef[:],
                         rth[:].unsqueeze(3).broadcast_to([128, BG, NJ, 3]))
    srd = wpool.tile([128, BG, NJ, 3], f32)
    nc.vector.tensor_mul(srd[:], rd[:],
                         sin[:].unsqueeze(3).broadcast_to([128, BG, NJ, 3]))
    # R = omc * (rd rd^T);  then += cos on diag, += sin*K off-diag
    ord_t = wpool.tile([128, BG, NJ, 3], f32)
    nc.vector.tensor_mul(ord_t[:], rd[:],
                         omc[:].unsqueeze(3).broadcast_to([128, BG, NJ, 3]))
    R = wpool.tile([128, BG, NJ, 3, 3], f32)
    nc.gpsimd.tensor_mul(
        R[:],
        rd[:].unsqueeze(4).broadcast_to([128, BG, NJ, 3, 3]),
        ord_t[:].unsqueeze(3).broadcast_to([128, BG, NJ, 3, 3]))
    Rf = R[:].rearrange("p g j m n -> p g j (m n)")
    diag = Rf[:, :, :, 0:9:4]
    nc.vector.tensor_add(diag, diag,
                         cos[:].unsqueeze(3).broadcast_to([128, BG, NJ, 3]))
    # K matrix: [[0,-z,y],[z,0,-x],[-y,x,0]] * sin
    for (mn, comp, sign) in ((1, 2, -1.0), (3, 2, 1.0), (2, 1, 1.0),
                             (6, 1, -1.0), (5, 0, -1.0), (7, 0, 1.0)):
        dst = Rf[:, :, :, mn:mn + 1].squeeze(3)
        src = srd[:, :, :, comp:comp + 1].squeeze(3)
        if sign > 0:
            nc.vector.tensor_add(dst, dst, src)
        else:
            nc.vector.tensor_sub(dst, dst, src)

    if STAGE < 2:
        stk.close()
        return
    if DEBUG:
        nc.sync.dma_start(tens["dbg_R"][:],
                          R[:].rearrange("p g j m n -> p (g j m n)"))

    # ---- pose-feature transposes: pfh [126, B], pfl [81, B] (rows (j,m,n)) ----
    pfh = wpool.tile([126, B], corr_sb_dt)
    pfl = wpool.tile([81, B], corr_sb_dt)
    pf_src = R[:].rearrange("p g j m n -> p g (j m n)")
    for bg in range(BG):
        for (lo, cnt, dstt, nI) in ((0, 126, pfh, negIh), (126, 81, pfl, negIl)):
            tp = ppool.tile([cnt, 128], f32, tag="ps1")
            nc.tensor.transpose(
                tp[:], pf_src[:, bg, 9 + lo: 9 + lo + cnt], ident[:])
            nc.scalar.activation(
                dstt[:, bg * 128:(bg + 1) * 128], tp[:], AF.Identity,
                bias=nI[:, 0:1])

    if STAGE < 2:
        stk.close()
        return
    # ---- J = betas_aug @ jdirs_aug -> [128,(bg,j,3)] ----
    J = wpool.tile([128, BG, NJ, 3], f32)
    for bg in range(BG):
        jp = ppool.tile([128, NJ * 3], f32, tag="ps1")
        nc.tensor.matmul(jp[:], bt[:, bg * 128:(bg + 1) * 128], jd[:])
        nc.scalar.copy(J[:, bg][:].rearrange("p j k -> p (j k)"), jp[:])

    if STAGE < 3:
        stk.close()
        return
    if DEBUG:
        nc.sync.dma_start(tens["dbg_pfh"][:], pfh[:])

    # ---- delta matmuls -> d4 [128,(bg,n,v)] bf16 plane-major ----
    d4 = wpool.tile([128, BG, 3, VS], bf16)
    for bg in range(BG):
        bsl = slice(bg * 128, (bg + 1) * 128)
        for vc in range(VS // VC):
            dp = ppool.tile([128, 3, 512], f32, tag="ps3")
            for ch in range(3):
                csl = slice(vc * 3 * VC + ch * VC, vc * 3 * VC + (ch + 1) * VC)
                mms = ((btr, sdt), (pfh, pdh), (pfl, pdl))
                for i, (lhs, rhs) in enumerate(mms):
                    nc.tensor.matmul(
                        dp[:, ch, 0:VC], lhs[:, bsl], rhs[:, csl],
                        start=(i == 0), stop=(i == len(mms) - 1))
            # psum cols are (v,n)-interleaved: chunk ch = verts [ch*144,+144)
            nc.scalar.copy(
                d4[:, bg, :, vc * VC:(vc + 1) * VC]
                .rearrange("p n (c v) -> p c n v", c=3),
                dp[:, :, 0:VC].rearrange("p c (v n) -> p c n v", n=3))

    if STAGE < 4:
        stk.close()
        return
    if DEBUG:
        nc.sync.dma_start(tens["dbg_J"][:], J[:].rearrange("p g j n -> p (g j n)"))
        nc.sync.dma_start(tens["dbg_d4"][:],
                          d4[:].rearrange("p g v n -> p (g v n)"))

    # ---- rel_joints (per kinematic-tree runs) ----
    rj = wpool.tile([128, BG, NJ, 3], f32)
    nc.gpsimd.tensor_copy(rj[:, :, 0], J[:, :, 0])
    nc.gpsimd.tensor_sub(rj[:, :, 1:4], J[:, :, 1:4],
                         J[:, :, 0:1].broadcast_to([128, BG, 3, 3]))
    nc.gpsimd.tensor_sub(rj[:, :, 4:13], J[:, :, 4:13], J[:, :, 1:10])
    nc.gpsimd.tensor_sub(rj[:, :, 13:15], J[:, :, 13:15],
                         J[:, :, 9:10].broadcast_to([128, BG, 2, 3]))
    nc.gpsimd.tensor_sub(rj[:, :, 15:18], J[:, :, 15:18], J[:, :, 12:15])
    nc.gpsimd.tensor_sub(rj[:, :, 18:24], J[:, :, 18:24], J[:, :, 16:22])

    if STAGE < 5:
        stk.close()
        return
    # ---- tmat: [R | rel_joints] 3x4 ----
    # tm is 4x4: rows 0..2 = [R | rel_joint], row 3 = [0,0,0,1]
    tm = wpool.tile([128, BG, NJ, 4, 4], f32)
    nc.gpsimd.memset(tm[:, :, :, 3, 0:3], 0.0)
    nc.gpsimd.memset(tm[:, :, :, 3, 3], 1.0)
    for bg in range(BG):
        nc.gpsimd.tensor_copy(tm[:, bg, :, 0:3, 0:3], R[:, bg])
    nc.gpsimd.tensor_copy(tm[:, :, :, 0:3, 3], rj[:])

    # ---- kinematic chain (3x4 affine composition) ----
    # j padded to 32 so the (n, j) transpose lands n-blocks at 32-aligned
    # partitions (matmul operand base must be 0/32/64); pad rows stay zero.
    ch_t = wpool.tile([128, BG, 32, 3, 4], f32)
    nc.gpsimd.memset(ch_t[:], 0.0)
    nc.gpsimd.tensor_copy(ch_t[:, :, 0, :, :], tm[:, :, 0, 0:3, :])
    tmpj = wpool.tile([128, BG, 3, 4, 4], f32)
    for bg in range(BG):
        for j in range(1, NJ):
            p = PAR[j]
            in0 = (ch_t[:, bg, p, :, :].unsqueeze(2)
                   .broadcast_to([128, 3, 4, 4]))
            in1 = (tm[:, bg, j].unsqueeze(1).transpose([0, 1, 3, 2])
                   .broadcast_to([128, 3, 4, 4]))
            nc.gpsimd.tensor_mul(tmpj[:, bg], in0, in1)
            nc.vector.reduce_sum(ch_t[:, bg, j, :, :], tmpj[:, bg],
                                 axis=mybir.AxisListType.X)

    if STAGE < 6:
        stk.close()
        return
    # posed joints output (before the rel subtraction)
    jstage = wpool.tile([128, BG, NJ, 3], f32)
    nc.vector.tensor_copy(jstage[:], ch_t[:, :, 0:NJ, :, 3])
    nc.sync.dma_start(
        joints_out[:].rearrange("p g (j k) -> p g j k", k=3), jstage[:])

    if STAGE < 7:
        stk.close()
        return
    # ---- rel transforms: translation -= chain_rot @ J ----
    tj_t = wpool.tile([128, BG, NJ, 3, 3], f32)
    tj = wpool.tile([128, BG, NJ, 3], f32)
    for bg in range(BG):
        nc.vector.tensor_mul(
            tj_t[:, bg], ch_t[:, bg, 0:NJ, :, 0:3],
            J[:, bg].unsqueeze(2).broadcast_to([128, NJ, 3, 3]))
        nc.vector.reduce_sum(tj[:, bg], tj_t[:, bg],
                             axis=mybir.AxisListType.X)
    nc.vector.tensor_sub(ch_t[:, :, 0:NJ, :, 3], ch_t[:, :, 0:NJ, :, 3], tj[:])

    if STAGE < 8:
        stk.close()
        return
    if DEBUG:
        nc.sync.dma_start(tens["dbg_ch"][:],
                          ch_t[:].rearrange("p g j m n -> p (g j m n)"))

    # ---- transposes -> relT1 [128=(n,j32), 12=(m,bg), 128] ----
    relT1 = wpool.tile([128, 12, 128], f32)
    relT1b = (wpool.tile([128, 12, 128], corr_sb_dt, tag="relT1b",
                         name="relT1b")
              if CORR_DT != "f32" else None)
    for bg in range(BG):
        for m in range(3):
            # contiguous (n, j) staging: transpose moving op needs 1 free dim
            cstg = spool.tile([128, 128], f32, tag="cstg")
            nc.gpsimd.tensor_copy(
                cstg[:].rearrange("p (n j) -> p n j", n=4),
                ch_t[:, bg, :, m, :].transpose([0, 2, 1]))
            tp = ppool.tile([128, 128], f32, tag="ps1")
            nc.tensor.transpose(tp[:], cstg[:], ident[:])
            nc.scalar.copy(relT1[:, m * 4 + bg], tp[:])
            if relT1b is not None:
                nc.scalar.copy(relT1b[:, m * 4 + bg], tp[:])

    if STAGE < 9:
        stk.close()
        return
    if DEBUG:
        nc.sync.dma_start(tens["dbg_relT1"][:],
                          relT1[:].rearrange("p c b -> p (c b)"))

    # ---- mv units: per (m, bg, vchunk): 3 M3 matmuls + term1 + mul-reduce ----
    relT3 = relT1b if relT1b is not None else relT1
    for bg in range(BG):
        for m in range(3):
            c = m * 4 + bg
            for vc in range(VS // VC):
                vsl = slice(vc * VC, (vc + 1) * VC)
                mv = ppool.tile([128, 3, 512], f32, tag="ps3")
                for n in range(3):
                    nc.tensor.matmul(
                        mv[:, n, 0:VC],
                        relT3[n * 32:n * 32 + NJ, c],
                        wT[n * 32:n * 32 + NJ, vsl])
                t1 = ppool.tile([128, VC], f32, tag="ps1")
                nc.tensor.matmul(t1[:], relT1[:, c], G[:, vsl])
                # evict the 3 M3 planes to bf16 (plane-major) on ACT
                mvs = spool.tile([128, 3, VC], bf16, tag="mvs")
                nc.scalar.copy(mvs[:], mv[:, :, 0:VC])
                # corr = sum_n plane_n * delta_n  (bf16 2x TT adds)
                tmpb = spool.tile([128, 3, VC], bf16, tag="tmpb")
                nc.vector.tensor_mul(tmpb[:], mvs[:], d4[:, bg, :, vsl])
                s01 = spool.tile([128, VC], bf16, tag="s01")
                nc.gpsimd.tensor_add(s01[:], tmpb[:, 0], tmpb[:, 1])
                s012 = spool.tile([128, VC], f32, tag="s012")
                nc.vector.tensor_add(s012[:], s01[:], tmpb[:, 2])
                vout = spool.tile([128, VC], f32, tag="vout")
                nc.vector.tensor_add(vout[:], s012[:], t1[:])
                nc.sync.dma_start(verts_out[c, :, vsl], vout[:])

    stk.close()


def _build():
    key = ("nc", CORR_DT, TMP_DT)
    if key in _CACHE:
        return _CACHE[key]
    import concourse.bacc as bacc
    import concourse.tile as tile
    import concourse.mybir as mybir

    dt = mybir.dt
    f32 = dt.float32
    cbdt = dt.bfloat16 if CORR_DT == "bf16" else f32

    nc = bacc.Bacc("TRN2", target_bir_lowering=False, debug=False)
    # extra const APs for ACT biases (preamble-style: memset + barrier)
    import numpy as _np
    for val in (1e-8, float(_np.pi / 2)):
        t = nc.alloc_sbuf_tensor(f"constx-{val}", [128, 1], dt.float32)
        nc.gpsimd.memset(t.ap(), val)
        nc.const_aps.aps[(dt.float32, val)] = t.ap()
    nc.all_engine_barrier()
    tens = {}

    def din(name, shape, dtype=f32):
        tens[name] = nc.dram_tensor(name, shape, dtype, kind="ExternalInput")[:]

    din("pose_t", [128, BG, NJ, 3])
    din("btaug", [NB + 1, B])
    din("jdirs", [NB + 1, NJ * 3])
    din("negI_hi", [126, 1])
    din("negI_lo", [81, 1])
    din("ident", [128, 128])
    din("G", [128, VS])
    din("wT", [88, VS], cbdt)
    din("pd_hi", [126, VS * 3], cbdt)
    din("pd_lo", [81, VS * 3], cbdt)
    din("sdT", [NB, VS * 3], cbdt)
    tens["verts_out"] = nc.dram_tensor(
        "verts_out", [12, 128, VS], f32, kind="ExternalOutput")[:]
    if os.environ.get("BODY_DEBUG", "0") != "0":
        for nm, shp in (("dbg_R", [128, 4 * NJ * 9]), ("dbg_J", [128, 4 * NJ * 3]),
                        ("dbg_ch", [128, 4 * 32 * 12]), ("dbg_d4", [128, BG * VS * 4]),
                        ("dbg_relT1", [128, 12 * 128]), ("dbg_pfh", [126, B])):
            tens[nm] = nc.dram_tensor(nm, shp, f32, kind="ExternalOutput")[:]
    tens["joints_out"] = nc.dram_tensor(
        "joints_out", [128, BG, NJ * 3], f32, kind="ExternalOutput")[:]

    with tile.TileContext(nc) as tc:
        _emit(nc, tc, tens)
    nc.compile()

    _CACHE[key] = (nc, tens)
    return nc, tens


def host_prep(betas, pose, v_template, shapedirs, posedirs, J_regressor,
              lbs_weights, parents):
    """Returns per-core input maps (numpy)."""
    import ml_dtypes

    f32 = np.float32
    cbdt = ml_dtypes.bfloat16 if CORR_DT == "bf16" else np.float32
    betas = np.asarray(betas, f32)
    pose = np.asarray(pose, f32)
    v_template = np.asarray(v_template, f32)
    shapedirs = np.asarray(shapedirs, f32)
    posedirs = np.asarray(posedirs, f32)
    J_regressor = np.asarray(J_regressor, f32)
    lbs_weights = np.asarray(lbs_weights, f32)

    pose_t = np.ascontiguousarray(
        pose.reshape(BG, 128, NJ, 3).transpose(1, 0, 2, 3))
    btaug = np.concatenate([betas.T, np.ones((1, B), f32)], axis=0)
    btaug = np.ascontiguousarray(btaug)
    jdirs_l = np.einsum("jv,vkl->ljk", J_regressor, shapedirs).reshape(NB, NJ * 3)
    jt = (J_regressor @ v_template).reshape(1, NJ * 3)
    jdirs = np.ascontiguousarray(np.concatenate([jdirs_l, jt], axis=0), f32)
    negI = np.zeros((207, 1), f32)
    for f in range(207):
        m, n = (f % 9) // 3, f % 3
        if m == n:
            negI[f, 0] = -1.0
    ident = np.eye(128, dtype=f32)

    # padded per-vertex params
    VP = NCORES * VS
    w_pad = np.zeros((VP, NJ), f32)
    w_pad[:V] = lbs_weights
    vt_pad = np.zeros((VP, 3), f32)
    vt_pad[:V] = v_template
    pd_pad = np.zeros((207, VP * 3), f32)
    pd_pad[:, :V * 3] = posedirs
    sd_pad = np.zeros((NB, VP * 3), f32)
    sd_pad[:, :V * 3] = shapedirs.transpose(2, 0, 1).reshape(NB, V * 3)

    in_maps = []
    for c in range(NCORES):
        vsl = slice(c * VS, (c + 1) * VS)
        csl = slice(c * VS * 3, (c + 1) * VS * 3)
        w_s = w_pad[vsl]                          # [VS, 24]
        vt_s = vt_pad[vsl]                        # [VS, 3]
        G = np.zeros((128, VS), f32)
        wT3 = np.zeros((88, VS), f32)
        for n in range(4):
            for j in range(NJ):
                G[n * 32 + j] = w_s[:, j] * (vt_s[:, n] if n < 3 else 1.0)
                if n < 3:
                    wT3[n * 32 + j] = w_s[:, j]
        in_maps.append({
            "pose_t": pose_t,
            "btaug": btaug,
            "jdirs": jdirs,
            "negI_hi": negI[:126],
            "negI_lo": negI[126:],
            "ident": ident,
            "G": G,
            "wT": wT3.astype(cbdt),
            "pd_hi": np.ascontiguousarray(pd_pad[:126, csl]).astype(cbdt),
            "pd_lo": np.ascontiguousarray(pd_pad[126:, csl]).astype(cbdt),
            "sdT": np.ascontiguousarray(sd_pad[:, csl]).astype(cbdt),
        })
    return in_maps


def assemble(results):
    """results: list of 8 dicts with verts_out/joints_out -> (verts, joints)."""
    verts = np.empty((B, V, 3), np.float32)
    for c in range(NCORES):
        vo = results[c]["verts_out"].reshape(3, BG, 128, VS)
        vo = vo.transpose(1, 2, 3, 0).reshape(B, VS, 3)
        n = min(VS, V - c * VS)
        verts[:, c * VS:c * VS + n] = vo[:, :n]
    jo = results[0]["joints_out"].reshape(128, BG, NJ, 3)
    joints = np.ascontiguousarray(jo.transpose(1, 0, 2, 3)).reshape(B, NJ, 3)
    return verts, joints


def kernel(**inputs):
    from concourse.bass_utils import run_bass_kernel_spmd

    nc, _ = _build()
    in_maps = host_prep(**inputs)
    res = run_bass_kernel_spmd(nc, in_maps, core_ids=list(range(NCORES)))
    return assemble(res.results)


def kernel_traced(**inputs):
    """Like kernel() but with NTFF profiling; returns (verts, joints, res)."""
    from concourse.bass_utils import run_bass_kernel_spmd

    nc, _ = _build()
    in_maps = host_prep(**inputs)
    res = run_bass_kernel_spmd(nc, in_maps, core_ids=list(range(NCORES)),
                               trace=True)
    verts, joints = assemble(res.results)
    return verts, joints, res


# revision 30
# speedup vs baseline: 1.4566x; 1.0644x over previous
"""SMPL body-model (B=512, V=6890, J=24) Bass kernel for 8 Trainium2 cores.

Strategy: vertex-shard V across the 8 cores (864 verts/core, zero-padded to
6912); every core computes the full batch B=512 for the small per-batch work
(Rodrigues, joint regression, kinematic chain) and its vertex slice for the
heavy per-vertex work.

Key algebraic restructure (avoids materializing [B,V,4,4] skinning mats):
  verts[b,v,m] = sum_{j,n} rel[b,j,m,n] * G[(n,j), v]              (term1)
               + sum_{j,n<3} rel[b,j,m,n] * w[v,j] * delta[b,v,n]  (corr)
  G[(n,j), v]  = w[v,j] * vt_h[n, v]   (batch-independent, host precomputed)
  delta[b,v,:] = blend-shape offsets + pose offsets (small magnitude)
so term1 is one K=96 fp32 matmul per output tile, and the correction runs
through K=24 matmuls (M3) + a per-vertex 4-wide multiply-reduce on DVE.
delta / M3 matmuls run as float32r (small-magnitude corrections only).
"""

import os
import numpy as np

B, V, NJ, NB = 512, 6890, 24, 10
NCORES = 8
VS = 864                    # vertex slice per core (8*864 = 6912 >= 6890)
VC = 432                    # vertex chunk (psum-bank aligned work unit)
BG = 4                      # batch groups of 128
PAR = [-1, 0, 0, 0, 1, 2, 3, 4, 5, 6, 7, 8, 9, 9, 9,
       12, 13, 14, 16, 17, 18, 19, 20, 21]

# matmul dtype for the correction path: "f32r" | "bf16" | "f32"
CORR_DT = os.environ.get("BODY_CORR_DT", "bf16")
STAGE = int(os.environ.get("BODY_STAGE", "99"))
DEBUG = bool(int(os.environ.get("BODY_DEBUG", "0")))
# dtype of the mul-reduce intermediate: bf16 (fast) or f32 (exact)
TMP_DT = os.environ.get("BODY_TMP_DT", "bf16")

_CACHE = {}


def _emit(nc, tc, tens):
    import concourse.bass as bass
    import concourse.mybir as mybir
    from concourse.bass import MemorySpace

    dt = mybir.dt
    f32 = dt.float32
    bf16 = dt.bfloat16
    AF = mybir.ActivationFunctionType
    ALU = mybir.AluOpType

    # dtype for correction-path matmul operand tiles; fp32r tiles must be
    # *produced* as fp32r (BIR verifier), so the tiles are declared f32r
    # and fp32 DRAM sources are bitcast at the DMA.
    corr_sb_dt = {"f32r": dt.float32r, "bf16": bf16, "f32": f32}[CORR_DT]

    def corr_src(ap):
        if CORR_DT == "f32r":
            return ap.bitcast(dt.float32r)
        return ap

    import contextlib
    stk = contextlib.ExitStack()
    cpool = stk.enter_context(tc.tile_pool(name="consts", bufs=1))
    wpool = stk.enter_context(tc.tile_pool(name="work", bufs=1))
    ppool = stk.enter_context(tc.tile_pool(name="ps", bufs=2, space="PSUM"))
    spool = stk.enter_context(tc.tile_pool(name="stream", bufs=3))

    # ---- load constants ----
    def load(name, shape, dtype=f32):
        t = cpool.tile(shape, dtype, tag=name)
        srcap = tens[name][:]
        if dtype == dt.float32r:
            srcap = srcap.bitcast(dt.float32r)
        nc.sync.dma_start(t[:], srcap)
        return t

    posef = load("pose_t", [128, BG, NJ, 3])
    bt = load("btaug", [NB + 1, B])
    btr = cpool.tile([NB, B], corr_sb_dt, tag="btr")
    if corr_sb_dt == bf16:
        nc.gpsimd.dma_start(btr[:], tens["btaug"][0:NB, :])
    else:
        nc.sync.dma_start(btr[:], corr_src(tens["btaug"][0:NB, :]))
    jd = load("jdirs", [NB + 1, NJ * 3])
    negIh = load("negI_hi", [126, 1])
    negIl = load("negI_lo", [81, 1])
    ident = load("ident", [128, 128])
    G = load("G", [128, VS])
    wT = load("wT", [88, VS], corr_sb_dt)
    pdh = load("pd_hi", [126, VS * 3], corr_sb_dt)
    pdl = load("pd_lo", [81, VS * 3], corr_sb_dt)
    sdt = load("sdT", [NB, VS * 3], corr_sb_dt)

    verts_out = tens["verts_out"]
    joints_out = tens["joints_out"]

    if STAGE < 1:
        stk.close()
        return
    # ---- Rodrigues: pose [128,(bg,j,3)] -> R [128,(bg,j,3,3)] ----
    rv1 = wpool.tile([128, BG, NJ, 3], f32)
    nc.scalar.add(rv1[:], posef[:], 1e-8)
    sq = wpool.tile([128, BG, NJ, 3], f32)
    nc.scalar.square(sq[:], rv1[:])
    th2 = wpool.tile([128, BG, NJ], f32)
    nc.vector.reduce_sum(th2[:], sq[:], axis=mybir.AxisListType.X)
    th = wpool.tile([128, BG, NJ], f32)
    nc.scalar.sqrt(th[:], th2[:])
    rth = wpool.tile([128, BG, NJ], f32)
    nc.vector.reciprocal(rth[:], th[:])
    sin = wpool.tile([128, BG, NJ], f32)
    nc.scalar.activation(sin[:], th[:], AF.Sin)
    cos = wpool.tile([128, BG, NJ], f32)
    nc.scalar.activation(cos[:], th[:], AF.Sin, bias=float(np.pi / 2))
    omc = wpool.tile([128, BG, NJ], f32)
    nc.scalar.activation(omc[:], cos[:], AF.Identity, bias=1.0, scale=-1.0)

    if STAGE < 1.5 and STAGE >= 1:
        pass
    rd = wpool.tile([128, BG, NJ, 3], f32)
    nc.vector.tensor_mul(rd[:], posef[:],
                         rth[:].unsqueeze(3).broadcast_to([128, BG, NJ, 3]))
    srd = wpool.tile([128, BG, NJ, 3], f32)
    nc.vector.tensor_mul(srd[:], rd[:],
                         sin[:].unsqueeze(3).broadcast_to([128, BG, NJ, 3]))
    # R = omc * (rd rd^T);  then += cos on diag, += sin*K off-diag
    ord_t = wpool.tile([128, BG, NJ, 3], f32)
    nc.vector.tensor_mul(ord_t[:], rd[:],
                         omc[:].unsqueeze(3).broadcast_to([128, BG, NJ, 3]))
    R = wpool.tile([128, BG, NJ, 3, 3], f32)
    nc.gpsimd.tensor_mul(
        R[:],
        rd[:].unsqueeze(4).broadcast_to([128, BG, NJ, 3, 3]),
        ord_t[:].unsqueeze(3).broadcast_to([128, BG, NJ, 3, 3]))
    Rf = R[:].rearrange("p g j m n -> p g j (m n)")
    diag = Rf[:, :, :, 0:9:4]
    nc.vector.tensor_add(diag, diag,
                         cos[:].unsqueeze(3).broadcast_to([128, BG, NJ, 3]))
    # K matrix: [[0,-z,y],[z,0,-x],[-y,x,0]] * sin
    for (mn, comp, sign) in ((1, 2, -1.0), (3, 2, 1.0), (2, 1, 1.0),
                             (6, 1, -1.0), (5, 0, -1.0), (7, 0, 1.0)):
        dst = Rf[:, :, :, mn:mn + 1].squeeze(3)
        src = srd[:, :, :, comp:comp + 1].squeeze(3)
        if sign > 0:
            nc.vector.tensor_add(dst, dst, src)
        else:
            nc.vector.tensor_sub(dst, dst, src)

    if STAGE < 2:
        stk.close()
        return
    if DEBUG:
        nc.sync.dma_start(tens["dbg_R"][:],
                          R[:].rearrange("p g j m n -> p (g j m n)"))

    # ---- pose-feature transposes: pfh [126, B], pfl [81, B] (rows (j,m,n)) ----
    pfh = wpool.tile([126, B], corr_sb_dt)
    pfl = wpool.tile([81, B], corr_sb_dt)
    pf_src = R[:].rearrange("p g j m n -> p g (j m n)")
    for bg in range(BG):
        for (lo, cnt, dstt, nI) in ((0, 126, pfh, negIh), (126, 81, pfl, negIl)):
            tp = ppool.tile([cnt, 128], f32, tag="ps1")
            nc.tensor.transpose(
                tp[:], pf_src[:, bg, 9 + lo: 9 + lo + cnt], ident[:])
            nc.scalar.activation(
                dstt[:, bg * 128:(bg + 1) * 128], tp[:], AF.Identity,
                bias=nI[:, 0:1])

    if STAGE < 2:
        stk.close()
        return
    # ---- J = betas_aug @ jdirs_aug -> [128,(bg,j,3)] ----
    J = wpool.tile([128, BG, NJ, 3], f32)
    for bg in range(BG):
        jp = ppool.tile([128, NJ * 3], f32, tag="ps1")
        nc.tensor.matmul(jp[:], bt[:, bg * 128:(bg + 1) * 128], jd[:])
        nc.scalar.copy(J[:, bg][:].rearrange("p j k -> p (j k)"), jp[:])

    if STAGE < 3:
        stk.close()
        return
    if DEBUG:
        nc.sync.dma_start(tens["dbg_pfh"][:], pfh[:])

    # ---- delta matmuls -> d4 [128,(bg,n,v)] bf16 plane-major ----
    d4 = wpool.tile([128, BG, 3, VS], bf16)
    for bg in range(BG):
        bsl = slice(bg * 128, (bg + 1) * 128)
        for vc in range(VS // VC):
            dp = ppool.tile([128, 3, 512], f32, tag="ps3")
            for ch in range(3):
                csl = slice(vc * 3 * VC + ch * VC, vc * 3 * VC + (ch + 1) * VC)
                mms = ((btr, sdt), (pfh, pdh), (pfl, pdl))
                for i, (lhs, rhs) in enumerate(mms):
                    nc.tensor.matmul(
                        dp[:, ch, 0:VC], lhs[:, bsl], rhs[:, csl],
                        start=(i == 0), stop=(i == len(mms) - 1))
            # psum cols are (v,n)-interleaved: chunk ch = verts [ch*144,+144)
            nc.scalar.copy(
                d4[:, bg, :, vc * VC:(vc + 1) * VC]
                .rearrange("p n (c v) -> p c n v", c=3),
                dp[:, :, 0:VC].rearrange("p c (v n) -> p c n v", n=3))

    if STAGE < 4:
        stk.close()
        return
    if DEBUG:
        nc.sync.dma_start(tens["dbg_J"][:], J[:].rearrange("p g j n -> p (g j n)"))
        nc.sync.dma_start(tens["dbg_d4"][:],
                          d4[:].rearrange("p g v n -> p (g v n)"))

    # ---- rel_joints (per kinematic-tree runs) ----
    rj = wpool.tile([128, BG, NJ, 3], f32)
    nc.gpsimd.tensor_copy(rj[:, :, 0], J[:, :, 0])
    nc.gpsimd.tensor_sub(rj[:, :, 1:4], J[:, :, 1:4],
                         J[:, :, 0:1].broadcast_to([128, BG, 3, 3]))
    nc.gpsimd.tensor_sub(rj[:, :, 4:13], J[:, :, 4:13], J[:, :, 1:10])
    nc.gpsimd.tensor_sub(rj[:, :, 13:15], J[:, :, 13:15],
                         J[:, :, 9:10].broadcast_to([128, BG, 2, 3]))
    nc.gpsimd.tensor_sub(rj[:, :, 15:18], J[:, :, 15:18], J[:, :, 12:15])
    nc.gpsimd.tensor_sub(rj[:, :, 18:24], J[:, :, 18:24], J[:, :, 16:22])

    if STAGE < 5:
        stk.close()
        return
    # ---- tmat: [R | rel_joints] 3x4 ----
    # tm is 4x4: rows 0..2 = [R | rel_joint], row 3 = [0,0,0,1]
    tm = wpool.tile([128, BG, NJ, 4, 4], f32)
    nc.gpsimd.memset(tm[:, :, :, 3, 0:3], 0.0)
    nc.gpsimd.memset(tm[:, :, :, 3, 3], 1.0)
    for bg in range(BG):
        nc.gpsimd.tensor_copy(tm[:, bg, :, 0:3, 0:3], R[:, bg])
    nc.gpsimd.tensor_copy(tm[:, :, :, 0:3, 3], rj[:])

    # ---- kinematic chain (3x4 affine composition) ----
    # j padded to 32 so the (n, j) transpose lands n-blocks at 32-aligned
    # partitions (matmul operand base must be 0/32/64); pad rows stay zero.
    ch_t = wpool.tile([128, BG, 32, 3, 4], f32)
    nc.gpsimd.memset(ch_t[:], 0.0)
    nc.gpsimd.tensor_copy(ch_t[:, :, 0, :, :], tm[:, :, 0, 0:3, :])
    tmpj = wpool.tile([128, BG, 3, 4, 4], f32)
    for bg in range(BG):
        for j in range(1, NJ):
            p = PAR[j]
            in0 = (ch_t[:, bg, p, :, :].unsqueeze(2)
                   .broadcast_to([128, 3, 4, 4]))
            in1 = (tm[:, bg, j].unsqueeze(1).transpose([0, 1, 3, 2])
                   .broadcast_to([128, 3, 4, 4]))
            nc.gpsimd.tensor_mul(tmpj[:, bg], in0, in1)
            nc.vector.reduce_sum(ch_t[:, bg, j, :, :], tmpj[:, bg],
                                 axis=mybir.AxisListType.X)

    if STAGE < 6:
        stk.close()
        return
    # posed joints output (before the rel subtraction)
    jstage = wpool.tile([128, BG, NJ, 3], f32)
    nc.vector.tensor_copy(jstage[:], ch_t[:, :, 0:NJ, :, 3])
    nc.sync.dma_start(
        joints_out[:].rearrange("p g (j k) -> p g j k", k=3), jstage[:])

    if STAGE < 7:
        stk.close()
        return
    # ---- rel transforms: translation -= chain_rot @ J ----
    tj_t = wpool.tile([128, BG, NJ, 3, 3], f32)
    tj = wpool.tile([128, BG, NJ, 3], f32)
    for bg in range(BG):
        nc.vector.tensor_mul(
            tj_t[:, bg], ch_t[:, bg, 0:NJ, :, 0:3],
            J[:, bg].unsqueeze(2).broadcast_to([128, NJ, 3, 3]))
        nc.vector.reduce_sum(tj[:, bg], tj_t[:, bg],
                             axis=mybir.AxisListType.X)
    nc.vector.tensor_sub(ch_t[:, :, 0:NJ, :, 3], ch_t[:, :, 0:NJ, :, 3], tj[:])

    if STAGE < 8:
        stk.close()
        return
    if DEBUG:
        nc.sync.dma_start(tens["dbg_ch"][:],
                          ch_t[:].rearrange("p g j m n -> p (g j m n)"))

    # ---- transposes -> relT1 [128=(n,j32), 12=(m,bg), 128] ----
    relT1 = wpool.tile([128, 12, 128], f32)
    relT1b = (wpool.tile([128, 12, 128], corr_sb_dt, tag="relT1b",
                         name="relT1b")
              if CORR_DT != "f32" else None)
    for bg in range(BG):
        for m in range(3):
            # contiguous (n, j) staging: transpose moving op needs 1 free dim
            cstg = spool.tile([128, 128], f32, tag="cstg")
            nc.gpsimd.tensor_copy(
                cstg[:].rearrange("p (n j) -> p n j", n=4),
                ch_t[:, bg, :, m, :].transpose([0, 2, 1]))
            tp = ppool.tile([128, 128], f32, tag="ps1")
            nc.tensor.transpose(tp[:], cstg[:], ident[:])
            nc.scalar.copy(relT1[:, m * 4 + bg], tp[:])
            if relT1b is not None:
                nc.scalar.copy(relT1b[:, m * 4 + bg], tp[:])

    if STAGE < 9:
        stk.close()
        return
    if DEBUG:
        nc.sync.dma_start(tens["dbg_relT1"][:],
                          relT1[:].rearrange("p c b -> p (c b)"))

    # ---- mv units: per (m, bg, vchunk): 3 M3 matmuls + term1 + mul-reduce ----
    relT3 = relT1b if relT1b is not None else relT1
    for bg in range(BG):
        for m in range(3):
            c = m * 4 + bg
            for vc in range(VS // VC):
                vsl = slice(vc * VC, (vc + 1) * VC)
                mv = ppool.tile([128, 3, 512], f32, tag="ps3")
                for n in range(3):
                    nc.tensor.matmul(
                        mv[:, n, 0:VC],
                        relT3[n * 32:n * 32 + NJ, c],
                        wT[n * 32:n * 32 + NJ, vsl])
                t1 = ppool.tile([128, VC], f32, tag="ps1")
                nc.tensor.matmul(t1[:], relT1[:, c], G[:, vsl])
                # evict the 3 M3 planes to bf16 (plane-major) on ACT
                mvs = spool.tile([128, 3, VC], bf16, tag="mvs")
                nc.scalar.copy(mvs[:], mv[:, :, 0:VC])
                # corr = sum_n plane_n * delta_n  (bf16 2x TT adds)
                tmpb = spool.tile([128, 3, VC], bf16, tag="tmpb")
                nc.vector.tensor_mul(tmpb[:], mvs[:], d4[:, bg, :, vsl])
                s01 = spool.tile([128, VC], bf16, tag="s01")
                nc.gpsimd.tensor_add(s01[:], tmpb[:, 0], tmpb[:, 1])
                s012 = spool.tile([128, VC], f32, tag="s012")
                nc.vector.tensor_add(s012[:], s01[:], tmpb[:, 2])
                vout = spool.tile([128, VC], f32, tag="vout")
                nc.vector.tensor_add(vout[:], s012[:], t1[:])
                nc.sync.dma_start(verts_out[c, :, vsl], vout[:])

    stk.close()


def _build():
    key = ("nc", CORR_DT, TMP_DT)
    if key in _CACHE:
        return _CACHE[key]
    import concourse.bacc as bacc
    import concourse.tile as tile
    import concourse.mybir as mybir

    dt = mybir.dt
    f32 = dt.float32
    cbdt = dt.bfloat16 if CORR_DT == "bf16" else f32

    nc = bacc.Bacc("TRN2", target_bir_lowering=False, debug=False)
    # extra const APs for ACT biases (preamble-style: memset + barrier)
    import numpy as _np
    for val in (1e-8, float(_np.pi / 2)):
        t = nc.alloc_sbuf_tensor(f"constx-{val}", [128, 1], dt.float32)
        nc.gpsimd.memset(t.ap(), val)
        nc.const_aps.aps[(dt.float32, val)] = t.ap()
    nc.all_engine_barrier()
    tens = {}

    def din(name, shape, dtype=f32):
        tens[name] = nc.dram_tensor(name, shape, dtype, kind="ExternalInput")[:]

    din("pose_t", [128, BG, NJ, 3])
    din("btaug", [NB + 1, B])
    din("jdirs", [NB + 1, NJ * 3])
    din("negI_hi", [126, 1])
    din("negI_lo", [81, 1])
    din("ident", [128, 128])
    din("G", [128, VS])
    din("wT", [88, VS], cbdt)
    din("pd_hi", [126, VS * 3], cbdt)
    din("pd_lo", [81, VS * 3], cbdt)
    din("sdT", [NB, VS * 3], cbdt)
    tens["verts_out"] = nc.dram_tensor(
        "verts_out", [12, 128, VS], f32, kind="ExternalOutput")[:]
    if os.environ.get("BODY_DEBUG", "0") != "0":
        for nm, shp in (("dbg_R", [128, 4 * NJ * 9]), ("dbg_J", [128, 4 * NJ * 3]),
                        ("dbg_ch", [128, 4 * 32 * 12]), ("dbg_d4", [128, BG * VS * 4]),
                        ("dbg_relT1", [128, 12 * 128]), ("dbg_pfh", [126, B])):
            tens[nm] = nc.dram_tensor(nm, shp, f32, kind="ExternalOutput")[:]
    tens["joints_out"] = nc.dram_tensor(
        "joints_out", [128, BG, NJ * 3], f32, kind="ExternalOutput")[:]

    with tile.TileContext(nc) as tc:
        _emit(nc, tc, tens)
    nc.compile()

    _CACHE[key] = (nc, tens)
    return nc, tens


def host_prep(betas, pose, v_template, shapedirs, posedirs, J_regressor,
              lbs_weights, parents):
    """Returns per-core input maps (numpy)."""
    import ml_dtypes

    f32 = np.float32
    cbdt = ml_dtypes.bfloat16 if CORR_DT == "bf16" else np.float32
    betas = np.asarray(betas, f32)
    pose = np.asarray(pose, f32)
    v_template = np.asarray(v_template, f32)
    shapedirs = np.asarray(shapedirs, f32)
    posedirs = np.asarray(posedirs, f32)
    J_regressor = np.asarray(J_regressor, f32)
    lbs_weights = np.asarray(lbs_weights, f32)

    pose_t = np.ascontiguousarray(
        pose.reshape(BG, 128, NJ, 3).transpose(1, 0, 2, 3))
    btaug = np.concatenate([betas.T, np.ones((1, B), f32)], axis=0)
    btaug = np.ascontiguousarray(btaug)
    jdirs_l = np.einsum("jv,vkl->ljk", J_regressor, shapedirs).reshape(NB, NJ * 3)
    jt = (J_regressor @ v_template).reshape(1, NJ * 3)
    jdirs = np.ascontiguousarray(np.concatenate([jdirs_l, jt], axis=0), f32)
    negI = np.zeros((207, 1), f32)
    for f in range(207):
        m, n = (f % 9) // 3, f % 3
        if m == n:
            negI[f, 0] = -1.0
    ident = np.eye(128, dtype=f32)

    # padded per-vertex params
    VP = NCORES * VS
    w_pad = np.zeros((VP, NJ), f32)
    w_pad[:V] = lbs_weights
    vt_pad = np.zeros((VP, 3), f32)
    vt_pad[:V] = v_template
    pd_pad = np.zeros((207, VP * 3), f32)
    pd_pad[:, :V * 3] = posedirs
    sd_pad = np.zeros((NB, VP * 3), f32)
    sd_pad[:, :V * 3] = shapedirs.transpose(2, 0, 1).reshape(NB, V * 3)

    in_maps = []
    for c in range(NCORES):
        vsl = slice(c * VS, (c + 1) * VS)
        csl = slice(c * VS * 3, (c + 1) * VS * 3)
        w_s = w_pad[vsl]                          # [VS, 24]
        vt_s = vt_pad[vsl]                        # [VS, 3]
        G = np.zeros((128, VS), f32)
        wT3 = np.zeros((88, VS), f32)
        for n in range(4):
            for j in range(NJ):
                G[n * 32 + j] = w_s[:, j] * (vt_s[:, n] if n < 3 else 1.0)
                if n < 3:
                    wT3[n * 32 + j] = w_s[:, j]
        in_maps.append({
            "pose_t": pose_t,
            "btaug": btaug,
            "jdirs": jdirs,
            "negI_hi": negI[:126],
            "negI_lo": negI[126:],
            "ident": ident,
            "G": G,
            "wT": wT3.astype(cbdt),
            "pd_hi": np.ascontiguousarray(pd_pad[:126, csl]).astype(cbdt),
            "pd_lo": np.ascontiguousarray(pd_pad[126:, csl]).astype(cbdt),
            "sdT": np.ascontiguousarray(sd_pad[:, csl]).astype(cbdt),
        })
    return in_maps


def assemble(results):
    """results: list of 8 dicts with verts_out/joints_out -> (verts, joints)."""
    verts = np.empty((B, V, 3), np.float32)
    for c in range(NCORES):
        vo = results[c]["verts_out"].reshape(3, BG, 128, VS)
        vo = vo.transpose(1, 2, 3, 0).reshape(B, VS, 3)
        n = min(VS, V - c * VS)
        verts[:, c * VS:c * VS + n] = vo[:, :n]
    jo = results[0]["joints_out"].reshape(128, BG, NJ, 3)
    joints = np.ascontiguousarray(jo.transpose(1, 0, 2, 3)).reshape(B, NJ, 3)
    return verts, joints


def kernel(**inputs):
    from concourse.bass_utils import run_bass_kernel_spmd

    nc, _ = _build()
    in_maps = host_prep(**inputs)
    res = run_bass_kernel_spmd(nc, in_maps, core_ids=list(range(NCORES)))
    return assemble(res.results)


def kernel_traced(**inputs):
    """Like kernel() but with NTFF profiling; returns (verts, joints, res)."""
    from concourse.bass_utils import run_bass_kernel_spmd

    nc, _ = _build()
    in_maps = host_prep(**inputs)
    res = run_bass_kernel_spmd(nc, in_maps, core_ids=list(range(NCORES)),
                               trace=True)
    verts, joints = assemble(res.results)
    return verts, joints, res
